# revision 52
# baseline (speedup 1.0000x reference)
"""Block-sparse attention (local + vertical-strided causal mask) on 8 TRN2 cores.

Sharding: one head per NeuronCore (H=8, n_cores=8).

Per-core device algorithm (head h, residue r = 7-h):
  The 4096x4096 score matrix is processed at 128x128 granularity:
  "pair" i = q block-rows (2i, 2i+1) (128 q tokens), "chunk" = 128 k tokens
  (2 mask blocks of 64). Local window -> chunks c in [i-8, i] of K itself;
  vertical-strided blocks -> host-gathered K_vert (6 blocks of 64, kb = 8j+r),
  processed as 3 chunks shared by all cores, with per-core validity applied
  as multiplicative 0/1 per-partition scalars (all-ones halves are skipped
  at build time). Vert visits are spread through the schedule (pair i's
  verts right before its closing chunk) to keep the DVE mask load even.

  S^T orientation: S^T[k,q] = kT_chunk.T @ qT_pair  (PE, bf16; sm_scale is
    folded into qT on the host). Window-start visits (c == i-8) only carry
    their valid first 64 q-cols when not tile-first.
  P^T = exp(S^T)                                     (ACT, one call per group)
  masks (triangle / window-start / vert validity)    (DVE)
  out[q,0:128] += P^T_chunk.T @ [V | 1]_chunk        (PE, PSUM-accumulated)
  col 128 of out = softmax denominator; normalize with per-partition
  reciprocal + tensor_scalar multiply into bf16, batched stores per
  pair-group ((partition, pair, d) DRAM layout; host unpermutes + casts).

Perf notes (why the structure looks like this):
  - Each dma_start costs ~650ns of issue time on its engine and ~2.5-4us of
    serialized per-instruction latency on its queue (~90 GB/s each), so
    inputs go out as a few large, need-ordered transfers balanced across
    the three DMA-capable queues (sync/scalar HWDGE + gpsimd SWDGE).
  - ~3.5us of throwaway matmuls on a memset tile warm the PE HAM clock gate
    (cold PE runs at 1.2 GHz instead of 2.4).
  - The steady-state window is Activation-bound (exp is 1 elem/cycle/lane
    at 1.2 GHz); PE and DVE run just below it.
"""

import numpy as np
import ml_dtypes

BF16 = ml_dtypes.bfloat16

H = 8
S = 4096
D = 128
BLK = 64
NB = S // BLK        # 64 block rows
NPAIR = NB // 2      # 32 row pairs
NVSLOT = 6           # usable vertical slots (kb = 8j + r <= 47)
NVC = NVSLOT // 2    # 3 vertical chunks
GROUP = 8            # PSUM staging slots per exp group (8 * 128 f32 = 2 banks)

NEG = -30000.0


def make_schedule():
    """Global ordered visit list. visit = (kind, idx, pair)
    kind "local": idx = chunk c (k blocks 2c, 2c+1), pairs i in [c, c+8]
    kind "vert":  idx = vc (K_vert slots 2vc, 2vc+1)
    Pair i's vert visits (all vc with 8*vc+8 <= i; chunks beyond that have no
    causally-valid content) are spread out: emitted right before pair i's
    closing local chunk c == i. This keeps the per-group DVE mask load even
    (clustered vert bursts back up the Vector engine, which stalls exp via
    ptt-buffer reuse)."""
    visits = []
    for c in range(NPAIR):
        for vc in range(NVC):
            if 8 * vc + 8 <= c:
                visits.append(("vert", vc, c))
        for i in range(c, min(c + 8, NPAIR - 1) + 1):
            visits.append(("local", c, i))
    return visits


def vert_visit_order():
    return [(vc_, i_) for (kind, vc_, i_) in make_schedule() if kind == "vert"]


def vert_half_all_valid(vc, i, hh):
    """True iff the validity scalar for this vert-visit half is 1.0 for every
    partition on every core (kb = 8*(2*vc + p//64) + r <= qb - 16 with
    qb = 2*i + hh, worst case r=7, slot 2*vc+1)."""
    return 8 * (2 * vc + 1) + 7 <= 2 * i + hh - 16


_PROGRAM = None


def _build_program(loop_n=None, ablate=(), pv_delay=3, group=GROUP, stage_bufs=2,
                   pt_bufs=None, exp_split=1, dma_split=16, qt_gpsimd=True,
                   ob_bufs=3, rd_bufs=4, vaug_gpsimd=False, store_sync=False):
    if pt_bufs is None:
        pt_bufs = pv_delay + 4
    """Build the SPMD program. loop_n: wrap the whole body (incl. input DMA)
    in an in-NEFF For loop with that trip count — used only for timing.
    ablate: subset of {"masks","pv","epi","exp"} — drop stages (timing only).
    pv_delay: groups of software-pipeline delay between S^T and PV.
    exp_split: number of ACT calls per group."""
    import contextlib
    import concourse.bass as bass
    import concourse.mybir as mybir
    import concourse.tile as tile
    from concourse import bacc

    fp32 = mybir.dt.float32
    bf16 = mybir.dt.bfloat16

    nc = bacc.Bacc("TRN2", target_bir_lowering=False, debug=False, num_devices=H)

    qt_d = nc.dram_tensor("qt", [D, S], bf16, kind="ExternalInput").ap()
    kt_d = nc.dram_tensor("kt", [D, S], bf16, kind="ExternalInput").ap()
    ktv_d = nc.dram_tensor("ktv", [D, NVSLOT * BLK], bf16, kind="ExternalInput").ap()
    vaug_d = nc.dram_tensor("vaug", [128, NPAIR, D + 1], bf16, kind="ExternalInput").ap()
    vvaug_d = nc.dram_tensor("vvaug", [128, NVC, D + 1], bf16, kind="ExternalInput").ap()
    # packed small tensors: one DMA each instead of four
    # pkf[:, 0] = unused, pkf[:, 1:97] = vs.reshape(128, 96)
    # (sm_scale is folded into qt on the host, so exp uses scale=1.0 imm)
    pkf_d = nc.dram_tensor("pkf", [128, 97], fp32, kind="ExternalInput").ap()
    # pkb[:, 0:128] = tri, pkb[:, 128:256] = mstart
    pkb_d = nc.dram_tensor("pkb", [128, 256], bf16, kind="ExternalInput").ap()
    # output in (partition, pair, d) layout, bf16; host unpermutes + casts
    o_d = nc.dram_tensor("o", [128, NPAIR, D], bf16, kind="ExternalOutput").ap()

    visits = make_schedule()
    # first/last visit index per pair
    first = {}
    last = {}
    for g, (kind, idx, i) in enumerate(visits):
        first.setdefault(i, g)
        last[i] = g
    # PSUM start_tensor_calc zeroes the full 2KB bank (zero-region), so only
    # the first matmul touching an oacc tile may carry start=True.
    tile_first = {}
    for g, (kind, idx, i) in enumerate(visits):
        tile_first.setdefault(i // 3, g)
    with tile.TileContext(nc) as tc:
        with (
            tc.tile_pool(name="big", bufs=1) as big,
            tc.tile_pool(name="stage", bufs=stage_bufs, space="PSUM") as stagep,
            tc.tile_pool(name="oacc", bufs=4, space="PSUM") as oaccp,
            tc.tile_pool(name="pt", bufs=pt_bufs) as ptp,
            tc.tile_pool(name="ob", bufs=ob_bufs) as obp,
            tc.tile_pool(name="rd", bufs=rd_bufs) as rdp,
        ):
            if loop_n is not None:
                loop_cm = tc.For_i(
                    0,
                    loop_n,
                    hint_engines=(
                        mybir.EngineType.PE,
                        mybir.EngineType.DVE,
                        mybir.EngineType.Activation,
                        mybir.EngineType.Pool,
                        mybir.EngineType.SP,
                    ),
                )
            else:
                loop_cm = contextlib.nullcontext()
            with loop_cm:
                _emit_body(nc, tc, locals(), frozenset(ablate),
                           pv_delay=pv_delay, group=group, exp_split=exp_split,
                           dma_split=dma_split, qt_gpsimd=qt_gpsimd,
                           vaug_gpsimd=vaug_gpsimd, store_sync=store_sync)
    nc.compile()
    return nc


def _emit_body(nc, tc, env, ablate=frozenset(), pv_delay=1, group=GROUP,
               exp_split=1, dma_split=16, qt_gpsimd=False, vaug_gpsimd=False,
               store_sync=False):
    GROUP = group
    import concourse.mybir as mybir

    fp32 = mybir.dt.float32
    bf16 = mybir.dt.bfloat16
    big, stagep, oaccp, ptp, obp, rdp = (
        env["big"], env["stagep"], env["oaccp"], env["ptp"], env["obp"], env["rdp"]
    )
    qt_d, kt_d, ktv_d, vaug_d, vvaug_d, pkf_d, pkb_d, o_d = (
        env["qt_d"], env["kt_d"], env["ktv_d"], env["vaug_d"], env["vvaug_d"],
        env["pkf_d"], env["pkb_d"], env["o_d"],
    )
    visits, first, last, tile_first = (
        env["visits"], env["first"], env["last"], env["tile_first"]
    )
    n_groups = (len(visits) + GROUP - 1) // GROUP
    if True:
        if True:
            qt = big.tile([D, S], bf16)
            kt = big.tile([D, S], bf16)
            ktv = big.tile([D, NVSLOT * BLK], bf16)
            vaug = big.tile([128, NPAIR, D + 1], bf16)
            vvaug = big.tile([128, NVC, D + 1], bf16)
            pkf = big.tile([128, 97], fp32)
            pkb = big.tile([128, 256], bf16)
            tri = pkb[:, 0:128]
            mstart = pkb[:, 128:256]

            def vs_ap(vi, hh):  # pkf col 1 + 2*vi + hh  (was vs[:, vi, hh])
                c0 = 1 + 2 * vi + hh
                return pkf[:, c0 : c0 + 1]

            # Input DMA: few big transfers, spread over the three DMA-capable
            # engines (sync/scalar HWDGE, gpsimd SWDGE), in first-need order.
            # Each dma_start costs ~650ns of engine issue time and each queue
            # sustains only ~90-150 GB/s, so balance bytes across all three
            # queues and order by first use.
            # Completion is serialized per queue at ~2.5-4us per DMA
            # *instruction* (fixed HWDGE/SWDGE overhead dominates for <1MB),
            # so the chain ORDER on each queue sets availability times.
            nc.sync.dma_start(out=kt[:, 0:512], in_=kt_d[:, 0:512])
            nc.sync.dma_start(out=pkb[:], in_=pkb_d[:])
            nc.sync.dma_start(out=kt[:, 512:1536], in_=kt_d[:, 512:1536])
            nc.sync.dma_start(out=kt[:, 1536:2560], in_=kt_d[:, 1536:2560])
            nc.sync.dma_start(out=kt[:, 2560:S], in_=kt_d[:, 2560:S])
            nc.scalar.dma_start(out=qt[:, 0:256], in_=qt_d[:, 0:256])
            nc.scalar.dma_start(out=qt[:, 256:512], in_=qt_d[:, 256:512])
            nc.scalar.dma_start(out=qt[:, 1536:2560], in_=qt_d[:, 1536:2560])
            nc.scalar.dma_start(out=qt[:, 2560:S], in_=qt_d[:, 2560:S])
            nc.scalar.dma_start(out=ktv[:], in_=ktv_d[:])
            nc.scalar.dma_start(out=vvaug[:], in_=vvaug_d[:])
            nc.gpsimd.dma_start(out=qt[:, 512:1536], in_=qt_d[:, 512:1536])
            nc.gpsimd.dma_start(out=vaug[:, 0:8], in_=vaug_d[:, 0:8])
            nc.gpsimd.dma_start(out=vaug[:, 8:20], in_=vaug_d[:, 8:20])
            nc.gpsimd.dma_start(out=vaug[:, 20:NPAIR], in_=vaug_d[:, 20:NPAIR])
            nc.gpsimd.dma_start(out=pkf[:], in_=pkf_d[:])

            # PE warm-up: ~3.5us of throwaway matmuls on a memset tile (no
            # DMA dependency) so the HAM clock gate opens (K=8/8) before the
            # real S^T stream begins; without this the first ~10 groups run
            # at 1.2 GHz.
            wsrc = big.tile([128, 256], bf16)
            nc.vector.memset(wsrc[:], 1.0)
            wtile = stagep.tile([128, GROUP * 128], fp32, tag="stage")
            # 24 also pads the PE queue through the ragged first groups so a
            # HAM MID window never sees enough idle to re-throttle
            for _w in range(24):
                nc.tensor.matmul(
                    wtile[:, 0:256], wsrc[:, 0:128], wsrc[:],
                    start=True, stop=True, skip_group_check=True,
                )

            oacc_tiles = {}  # pair-group (i//3) -> psum tile [128, 3, 129]
            v_idx = 0  # running vertical-visit index (matches host vs layout)
            pending_pv = []  # software pipeline: PV of group gi-d emitted
            # after S^T of group gi so PE streams while ACT/DVE process gi-d

            last_pg = (NPAIR - 1) // 3
            for gi in range(n_groups):
                gvis = visits[gi * GROUP : (gi + 1) * GROUP]
                n = len(gvis)
                stage = stagep.tile([128, GROUP * 128], fp32, tag="stage")
                ptt = ptp.tile([128, GROUP * 128], bf16, tag="pt")

                # Per-visit stage widths: window-start visits (idx == i-8)
                # only have valid content in their first 64 q-cols (qblock
                # 2i); use a 64-wide slot unless the visit is tile-first (a
                # 64-partition PV start=True might not zero the whole bank).
                widths = []
                for s, (kind, idx, i) in enumerate(gvis):
                    g = gi * GROUP + s
                    narrow = (
                        kind == "local" and idx == i - 8
                        and tile_first[i // 3] != g
                    )
                    widths.append(64 if narrow else 128)
                offs = []
                off = 0
                for w in widths:
                    if w == 128 and off // 512 != (off + 127) // 512:
                        off = (off // 512 + 1) * 512  # don't straddle a bank
                    offs.append(off)
                    off += w
                gw = off

                # --- S^T matmuls, batched over runs of consecutive pairs
                # sharing one k-chunk, split at PSUM bank (512 f32) bounds.
                # start=True only on the first run per bank (bank zero-region).
                s = 0
                seen_banks = set()
                while s < n:
                    kind, idx, i0 = gvis[s]
                    e = s + 1
                    # group 0: cap runs at 2 visits so the first matmul only
                    # needs qt[0:256] (its DMA completes ~1.3us earlier than
                    # the full 512-col piece)
                    run_cap = 2 if gi == 0 else GROUP
                    if widths[s] == 128:
                        while (
                            e < n
                            and e - s < run_cap
                            and widths[e] == 128
                            and gvis[e][0] == kind
                            and gvis[e][1] == idx
                            and gvis[e][2] == gvis[e - 1][2] + 1
                            and offs[e] == offs[e - 1] + widths[e - 1]
                            and offs[e] + 128 <= (offs[s] // 512 + 1) * 512
                        ):
                            e += 1
                    d0 = offs[s]
                    d1 = offs[e - 1] + widths[e - 1]
                    lhsT = (
                        kt[:, idx * 128 : (idx + 1) * 128]
                        if kind == "local"
                        else ktv[:, idx * 128 : (idx + 1) * 128]
                    )
                    banks = range(d0 // 512, (d1 - 1) // 512 + 1)
                    nc.tensor.matmul(
                        stage[:, d0:d1],
                        lhsT,
                        qt[:, i0 * 128 : i0 * 128 + (d1 - d0)],
                        start=any(b not in seen_banks for b in banks),
                        stop=True,
                        skip_group_check=True,
                    )
                    seen_banks.update(banks)
                    s = e

                if len(pending_pv) >= pv_delay:
                    pending_pv.pop(0)()

                # --- exp for the group
                if "exp" not in ablate:
                    nc.scalar.activation(
                        out=ptt[:, 0:gw],
                        in_=stage[:, 0:gw],
                        func=mybir.ActivationFunctionType.Exp,
                    )

                # --- masks
                for s, (kind, idx, i) in enumerate(gvis):
                    if "masks" in ablate:
                        if kind == "vert":
                            v_idx += 1
                        continue
                    off, w = offs[s], widths[s]
                    sl = slice(off, off + w)
                    if kind == "local" and idx == i:
                        nc.vector.tensor_mul(ptt[:, sl], ptt[:, sl], tri)
                    elif kind == "local" and idx == i - 8:
                        nc.vector.tensor_mul(
                            ptt[:, sl], ptt[:, sl],
                            mstart[:, 0:64] if w == 64 else mstart,
                        )
                    elif kind == "vert":
                        for hh in range(2):
                            # skip halves whose validity scalar is 1.0 on
                            # every core (multiply-by-one is a no-op)
                            if vert_half_all_valid(idx, i, hh):
                                continue
                            hsl = slice(off + hh * 64, off + (hh + 1) * 64)
                            nc.vector.tensor_scalar_mul(
                                ptt[:, hsl], ptt[:, hsl], vs_ap(v_idx, hh)
                            )
                        v_idx += 1

                # --- PV matmuls + epilogue (deferred one group)
                def make_pv(gi, gvis, ptt, offs, widths):
                    def emit_pv():
                        if "pv" in ablate:
                            return
                        for s, (kind, idx, i) in enumerate(gvis):
                            g = gi * GROUP + s
                            pg = i // 3
                            if pg not in oacc_tiles:
                                oacc_tiles[pg] = oaccp.tile(
                                    [128, 3, D + 1], fp32, tag="oacc", name=f"oacc{pg}"
                                )
                            oacc = oacc_tiles[pg]
                            rhs = vaug[:, idx] if kind == "local" else vvaug[:, idx]
                            off, w = offs[s], widths[s]
                            nc.tensor.matmul(
                                oacc[:, i % 3] if w == 128
                                else oacc[0:64, i % 3],
                                ptt[:, off : off + w],
                                rhs,
                                start=(g == tile_first[i // 3]),
                                stop=(g == last[i]),
                                skip_group_check=True,
                            )
                            # epilogue once per oacc tile (after its last
                            # pair closes): a single DVE read of the PSUM
                            # bank, so PE's later PV writes to that bank are
                            # never serialized against mid-tile DVE reads.
                            # The final pair-group instead closes per pair so
                            # the very last store leaves as early as possible.
                            pg_pairs = [p for p in (3 * pg, 3 * pg + 1, 3 * pg + 2)
                                        if p < NPAIR]
                            if "epi" in ablate:
                                continue
                            if pg == last_pg and g == last[i]:
                                jj = i % 3
                                osb = obp.tile([128, D + 1], fp32, tag="osb1")
                                nc.vector.tensor_copy(osb[:], oacc[:, jj])
                                rd = rdp.tile([128, 1], fp32, tag="rd")
                                nc.vector.reciprocal(rd[:], osb[:, D : D + 1])
                                ob1 = obp.tile([128, 1, D], bf16, tag="ob1")
                                nc.vector.tensor_scalar_mul(
                                    ob1[:, 0], osb[:, 0:D], rd[:]
                                )
                                st_eng = nc.sync if i % 2 == 0 else nc.scalar
                                st_eng.dma_start(
                                    out=o_d[:, i : i + 1, :], in_=ob1[:]
                                )
                            elif (
                                pg != last_pg
                                and i == pg_pairs[-1]
                                and g == last[i]
                            ):
                                # single PSUM read frees the oacc bank fast
                                # (the next pg's PV start=True waits on it)
                                osb = obp.tile([128, 3, D + 1], fp32, tag="osb")
                                nc.vector.tensor_copy(osb[:], oacc[:])
                                ob3 = obp.tile([128, 3, D], bf16, tag="ob3")
                                for jj, pp in enumerate(pg_pairs):
                                    rd = rdp.tile([128, 1], fp32, tag="rd")
                                    nc.vector.reciprocal(
                                        rd[:], osb[:, jj, D : D + 1]
                                    )
                                    nc.vector.tensor_scalar_mul(
                                        ob3[:, jj], osb[:, jj, 0:D], rd[:]
                                    )
                                # one batched store per pair-group (bf16,
                                # (partition, pair, d) DRAM layout) on the
                                # two HWDGE queues, which are idle after the
                                # input loads and drain faster than SWDGE
                                p0 = pg_pairs[0]
                                st_eng = nc.sync if pg % 2 == 0 else nc.scalar
                                st_eng.dma_start(
                                    out=o_d[:, p0 : p0 + len(pg_pairs), :],
                                    in_=ob3[:, 0 : len(pg_pairs)],
                                )
                    return emit_pv

                pending_pv.append(make_pv(gi, gvis, ptt, offs, widths))
            for f in pending_pv:
                f()


def _get_program():
    global _PROGRAM
    if _PROGRAM is None:
        _PROGRAM = _build_program()
    return _PROGRAM


def _host_inputs(q, k, v, sm_scale):
    """Per-core input dicts (host-side shard + layout)."""
    q = np.asarray(q, dtype=np.float32)
    k = np.asarray(k, dtype=np.float32)
    v = np.asarray(v, dtype=np.float32)
    smv = float(np.asarray(sm_scale, dtype=np.float32))

    tri = np.zeros((128, 128), dtype=BF16)
    p = np.arange(128)
    tri[p[:, None] <= p[None, :]] = BF16(1.0)
    mstart = np.zeros((128, 128), dtype=BF16)
    mstart[64:, :64] = BF16(1.0)
    smsc = np.full((128, 1), smv, dtype=np.float32)

    vorder = vert_visit_order()
    ins = []
    for h in range(H):
        r = 7 - h
        qh, kh, vh = q[0, h], k[0, h], v[0, h]
        # fold sm_scale into q so the device exp needs no scale operand
        qt = np.ascontiguousarray(qh.T * smv).astype(BF16)
        kt = np.ascontiguousarray(kh.T).astype(BF16)
        vblocks = [8 * j + r for j in range(NVSLOT)]
        kv = np.concatenate([kh[b * BLK : (b + 1) * BLK] for b in vblocks], axis=0)
        ktv = np.ascontiguousarray(kv.T).astype(BF16)
        vaug = np.concatenate(
            [vh, np.ones((S, 1), np.float32)], axis=1
        ).astype(BF16)  # [4096, 129]
        vaug = np.ascontiguousarray(
            vaug.reshape(NPAIR, 128, D + 1).transpose(1, 0, 2)
        )  # [128, 32, 129]
        vv = np.concatenate([vh[b * BLK : (b + 1) * BLK] for b in vblocks], axis=0)
        vvaug = np.concatenate([vv, np.ones((NVSLOT * BLK, 1), np.float32)], axis=1)
        vvaug = np.ascontiguousarray(
            vvaug.astype(BF16).reshape(NVC, 128, D + 1).transpose(1, 0, 2)
        )  # [128, 3, 129]

        vsc = np.zeros((128, 48, 2), dtype=np.float32)
        for vi, (vc, i) in enumerate(vorder):
            for hh in range(2):
                qb = 2 * i + hh
                slot = 2 * vc + (p >= 64).astype(np.int64)  # per-partition slot
                kb = 8 * slot + r
                vsc[:, vi, hh] = (kb <= qb - 16).astype(np.float32)
        pkf = np.concatenate([smsc, vsc.reshape(128, 96)], axis=1)
        pkb = np.concatenate([tri, mstart], axis=1)
        ins.append(
            dict(
                qt=qt, kt=kt, ktv=ktv, vaug=vaug, vvaug=vvaug,
                pkf=pkf, pkb=pkb,
            )
        )
    return ins


def kernel(q, k, v, sm_scale):
    from concourse.bass_utils import run_bass_kernel_spmd

    nc = _get_program()
    ins = _host_inputs(q, k, v, sm_scale)
    res = run_bass_kernel_spmd(nc, ins, core_ids=list(range(H)))
    # o is bf16 [128, NPAIR, D] with (partition, pair, d) layout; unpermute
    out = np.stack(
        [
            np.ascontiguousarray(
                res.results[h]["o"].transpose(1, 0, 2)
            ).reshape(S, D)
            for h in range(H)
        ],
        axis=0,
    )[None]
    return out.astype(np.float32)



# revision 53
# speedup vs baseline: 1.0272x; 1.0272x over previous
"""Block-sparse attention (local + vertical-strided causal mask) on 8 TRN2 cores.

Sharding: one head per NeuronCore (H=8, n_cores=8).

Per-core device algorithm (head h, residue r = 7-h):
  The 4096x4096 score matrix is processed at 128x128 granularity:
  "pair" i = q block-rows (2i, 2i+1) (128 q tokens), "chunk" = 128 k tokens
  (2 mask blocks of 64). Local window -> chunks c in [i-8, i] of K itself;
  vertical-strided blocks -> host-gathered K_vert (6 blocks of 64, kb = 8j+r),
  processed as 3 chunks shared by all cores, with per-core validity applied
  as multiplicative 0/1 per-partition scalars (all-ones halves are skipped
  at build time). Vert visits are spread through the schedule (pair i's
  verts right before its closing chunk) to keep the DVE mask load even.

  S^T orientation: S^T[k,q] = kT_chunk.T @ qT_pair  (PE, bf16; sm_scale is
    folded into qT on the host). Window-start visits (c == i-8) only carry
    their valid first 64 q-cols when not tile-first.
  P^T = exp(S^T)                                     (ACT, one call per group)
  masks (triangle / window-start / vert validity)    (DVE)
  out[q,0:128] += P^T_chunk.T @ [V | 1]_chunk        (PE, PSUM-accumulated)
  col 128 of out = softmax denominator; normalize with per-partition
  reciprocal + tensor_scalar multiply into bf16, batched stores per
  pair-group ((partition, pair, d) DRAM layout; host unpermutes + casts).

Perf notes (why the structure looks like this):
  - Each dma_start costs ~650ns of issue time on its engine and ~2.5-4us of
    serialized per-instruction latency on its queue (~90 GB/s each), so
    inputs go out as a few large, need-ordered transfers balanced across
    the three DMA-capable queues (sync/scalar HWDGE + gpsimd SWDGE).
  - ~3.5us of throwaway matmuls on a memset tile warm the PE HAM clock gate
    (cold PE runs at 1.2 GHz instead of 2.4).
  - The steady-state window is Activation-bound (exp is 1 elem/cycle/lane
    at 1.2 GHz); PE and DVE run just below it.
"""

import numpy as np
import ml_dtypes

BF16 = ml_dtypes.bfloat16

H = 8
S = 4096
D = 128
BLK = 64
NB = S // BLK        # 64 block rows
NPAIR = NB // 2      # 32 row pairs
NVSLOT = 6           # usable vertical slots (kb = 8j + r <= 47)
NVC = NVSLOT // 2    # 3 vertical chunks
GROUP = 8            # PSUM staging slots per exp group (8 * 128 f32 = 2 banks)

NEG = -30000.0


def make_schedule():
    """Global ordered visit list. visit = (kind, idx, pair)
    kind "local": idx = chunk c (k blocks 2c, 2c+1), pairs i in [c, c+8]
    kind "vert":  idx = vc (K_vert slots 2vc, 2vc+1)
    Pair i's vert visits (all vc with 8*vc+8 <= i; chunks beyond that have no
    causally-valid content) are spread out: emitted right before pair i's
    closing local chunk c == i. This keeps the per-group DVE mask load even
    (clustered vert bursts back up the Vector engine, which stalls exp via
    ptt-buffer reuse)."""
    visits = []
    for c in range(NPAIR):
        for vc in range(NVC):
            if 8 * vc + 8 <= c:
                visits.append(("vert", vc, c))
        for i in range(c, min(c + 8, NPAIR - 1) + 1):
            visits.append(("local", c, i))
    return visits


def vert_visit_order():
    return [(vc_, i_) for (kind, vc_, i_) in make_schedule() if kind == "vert"]


def vert_half_all_valid(vc, i, hh):
    """True iff the validity scalar for this vert-visit half is 1.0 for every
    partition on every core (kb = 8*(2*vc + p//64) + r <= qb - 16 with
    qb = 2*i + hh, worst case r=7, slot 2*vc+1)."""
    return 8 * (2 * vc + 1) + 7 <= 2 * i + hh - 16


_PROGRAM = None


def _build_program(loop_n=None, ablate=(), pv_delay=3, group=GROUP, stage_bufs=2,
                   pt_bufs=None, exp_split=1, dma_split=16, qt_gpsimd=True,
                   ob_bufs=3, rd_bufs=4, vaug_gpsimd=False, store_sync=False):
    if pt_bufs is None:
        pt_bufs = pv_delay + 4
    """Build the SPMD program. loop_n: wrap the whole body (incl. input DMA)
    in an in-NEFF For loop with that trip count — used only for timing.
    ablate: subset of {"masks","pv","epi","exp"} — drop stages (timing only).
    pv_delay: groups of software-pipeline delay between S^T and PV.
    exp_split: number of ACT calls per group."""
    import contextlib
    import concourse.bass as bass
    import concourse.mybir as mybir
    import concourse.tile as tile
    from concourse import bacc

    fp32 = mybir.dt.float32
    bf16 = mybir.dt.bfloat16

    nc = bacc.Bacc("TRN2", target_bir_lowering=False, debug=False, num_devices=H)

    qt_d = nc.dram_tensor("qt", [D, S], bf16, kind="ExternalInput").ap()
    kt_d = nc.dram_tensor("kt", [D, S], bf16, kind="ExternalInput").ap()
    ktv_d = nc.dram_tensor("ktv", [D, NVSLOT * BLK], bf16, kind="ExternalInput").ap()
    vaug_d = nc.dram_tensor("vaug", [128, NPAIR, D + 1], bf16, kind="ExternalInput").ap()
    vvaug_d = nc.dram_tensor("vvaug", [128, NVC, D + 1], bf16, kind="ExternalInput").ap()
    # packed small tensors: one DMA each instead of four
    # pkf[:, 0] = unused, pkf[:, 1:97] = vs.reshape(128, 96)
    # (sm_scale is folded into qt on the host, so exp uses scale=1.0 imm)
    pkf_d = nc.dram_tensor("pkf", [128, 97], fp32, kind="ExternalInput").ap()
    # pkb[:, 0:128] = tri, pkb[:, 128:256] = mstart
    pkb_d = nc.dram_tensor("pkb", [128, 256], bf16, kind="ExternalInput").ap()
    # output in (partition, pair, d) layout, bf16; host unpermutes + casts
    o_d = nc.dram_tensor("o", [128, NPAIR, D], bf16, kind="ExternalOutput").ap()

    visits = make_schedule()
    # first/last visit index per pair
    first = {}
    last = {}
    for g, (kind, idx, i) in enumerate(visits):
        first.setdefault(i, g)
        last[i] = g
    # PSUM start_tensor_calc zeroes the full 2KB bank (zero-region), so only
    # the first matmul touching an oacc tile may carry start=True.
    tile_first = {}
    for g, (kind, idx, i) in enumerate(visits):
        tile_first.setdefault(i // 3, g)
    with tile.TileContext(nc) as tc:
        with (
            tc.tile_pool(name="big", bufs=1) as big,
            tc.tile_pool(name="stage", bufs=stage_bufs, space="PSUM") as stagep,
            tc.tile_pool(name="oacc", bufs=4, space="PSUM") as oaccp,
            tc.tile_pool(name="pt", bufs=pt_bufs) as ptp,
            tc.tile_pool(name="ob", bufs=ob_bufs) as obp,
            tc.tile_pool(name="rd", bufs=rd_bufs) as rdp,
        ):
            if loop_n is not None:
                loop_cm = tc.For_i(
                    0,
                    loop_n,
                    hint_engines=(
                        mybir.EngineType.PE,
                        mybir.EngineType.DVE,
                        mybir.EngineType.Activation,
                        mybir.EngineType.Pool,
                        mybir.EngineType.SP,
                    ),
                )
            else:
                loop_cm = contextlib.nullcontext()
            with loop_cm:
                _emit_body(nc, tc, locals(), frozenset(ablate),
                           pv_delay=pv_delay, group=group, exp_split=exp_split,
                           dma_split=dma_split, qt_gpsimd=qt_gpsimd,
                           vaug_gpsimd=vaug_gpsimd, store_sync=store_sync)
    nc.compile()
    return nc


def _emit_body(nc, tc, env, ablate=frozenset(), pv_delay=1, group=GROUP,
               exp_split=1, dma_split=16, qt_gpsimd=False, vaug_gpsimd=False,
               store_sync=False):
    GROUP = group
    import concourse.mybir as mybir

    fp32 = mybir.dt.float32
    bf16 = mybir.dt.bfloat16
    big, stagep, oaccp, ptp, obp, rdp = (
        env["big"], env["stagep"], env["oaccp"], env["ptp"], env["obp"], env["rdp"]
    )
    qt_d, kt_d, ktv_d, vaug_d, vvaug_d, pkf_d, pkb_d, o_d = (
        env["qt_d"], env["kt_d"], env["ktv_d"], env["vaug_d"], env["vvaug_d"],
        env["pkf_d"], env["pkb_d"], env["o_d"],
    )
    visits, first, last, tile_first = (
        env["visits"], env["first"], env["last"], env["tile_first"]
    )
    n_groups = (len(visits) + GROUP - 1) // GROUP
    if True:
        if True:
            qt = big.tile([D, S], bf16)
            kt = big.tile([D, S], bf16)
            ktv = big.tile([D, NVSLOT * BLK], bf16)
            vaug = big.tile([128, NPAIR, D + 1], bf16)
            vvaug = big.tile([128, NVC, D + 1], bf16)
            pkf = big.tile([128, 97], fp32)
            pkb = big.tile([128, 256], bf16)
            tri = pkb[:, 0:128]
            mstart = pkb[:, 128:256]

            def vs_ap(vi, hh):  # pkf col 1 + 2*vi + hh  (was vs[:, vi, hh])
                c0 = 1 + 2 * vi + hh
                return pkf[:, c0 : c0 + 1]

            # Input DMA: few big transfers, spread over the three DMA-capable
            # engines (sync/scalar HWDGE, gpsimd SWDGE), in first-need order.
            # Each dma_start costs ~650ns of engine issue time and each queue
            # sustains only ~90-150 GB/s, so balance bytes across all three
            # queues and order by first use.
            # Completion is serialized per queue at ~2.5-4us per DMA
            # *instruction* (fixed HWDGE/SWDGE overhead dominates for <1MB),
            # so the chain ORDER on each queue sets availability times.
            nc.sync.dma_start(out=kt[:, 0:512], in_=kt_d[:, 0:512])
            nc.sync.dma_start(out=pkb[:], in_=pkb_d[:])
            nc.sync.dma_start(out=kt[:, 512:1536], in_=kt_d[:, 512:1536])
            nc.sync.dma_start(out=kt[:, 1536:2560], in_=kt_d[:, 1536:2560])
            nc.sync.dma_start(out=kt[:, 2560:S], in_=kt_d[:, 2560:S])
            nc.scalar.dma_start(out=qt[:, 0:512], in_=qt_d[:, 0:512])
            nc.scalar.dma_start(out=qt[:, 1536:2560], in_=qt_d[:, 1536:2560])
            nc.scalar.dma_start(out=qt[:, 2560:S], in_=qt_d[:, 2560:S])
            nc.scalar.dma_start(out=ktv[:], in_=ktv_d[:])
            nc.scalar.dma_start(out=vvaug[:], in_=vvaug_d[:])
            nc.gpsimd.dma_start(out=qt[:, 512:1536], in_=qt_d[:, 512:1536])
            nc.gpsimd.dma_start(out=vaug[:, 0:8], in_=vaug_d[:, 0:8])
            nc.gpsimd.dma_start(out=vaug[:, 8:20], in_=vaug_d[:, 8:20])
            nc.gpsimd.dma_start(out=vaug[:, 20:NPAIR], in_=vaug_d[:, 20:NPAIR])
            nc.gpsimd.dma_start(out=pkf[:], in_=pkf_d[:])

            # PE warm-up: ~3.5us of throwaway matmuls on a memset tile (no
            # DMA dependency) so the HAM clock gate opens (K=8/8) before the
            # real S^T stream begins; without this the first ~10 groups run
            # at 1.2 GHz.
            wsrc = big.tile([128, 256], bf16)
            nc.vector.memset(wsrc[:], 1.0)
            wtile = stagep.tile([128, GROUP * 128], fp32, tag="stage")
            for _w in range(24):
                nc.tensor.matmul(
                    wtile[:, 0:256], wsrc[:, 0:128], wsrc[:],
                    start=True, stop=True, skip_group_check=True,
                )

            oacc_tiles = {}  # pair-group (i//3) -> psum tile [128, 3, 129]
            v_idx = 0  # running vertical-visit index (matches host vs layout)
            pending_pv = []  # software pipeline: PV of group gi-d emitted
            # after S^T of group gi so PE streams while ACT/DVE process gi-d

            last_pg = (NPAIR - 1) // 3
            for gi in range(n_groups):
                gvis = visits[gi * GROUP : (gi + 1) * GROUP]
                n = len(gvis)
                stage = stagep.tile([128, GROUP * 128], fp32, tag="stage")
                ptt = ptp.tile([128, GROUP * 128], bf16, tag="pt")

                # Per-visit stage widths: window-start visits (idx == i-8)
                # only have valid content in their first 64 q-cols (qblock
                # 2i); use a 64-wide slot unless the visit is tile-first (a
                # 64-partition PV start=True might not zero the whole bank).
                widths = []
                for s, (kind, idx, i) in enumerate(gvis):
                    g = gi * GROUP + s
                    narrow = (
                        kind == "local" and idx == i - 8
                        and tile_first[i // 3] != g
                    )
                    widths.append(64 if narrow else 128)
                offs = []
                off = 0
                for w in widths:
                    if w == 128 and off // 512 != (off + 127) // 512:
                        off = (off // 512 + 1) * 512  # don't straddle a bank
                    offs.append(off)
                    off += w
                gw = off

                # --- S^T matmuls, batched over runs of consecutive pairs
                # sharing one k-chunk, split at PSUM bank (512 f32) bounds.
                # start=True only on the first run per bank (bank zero-region).
                s = 0
                seen_banks = set()
                while s < n:
                    kind, idx, i0 = gvis[s]
                    e = s + 1
                    if widths[s] == 128:
                        while (
                            e < n
                            and widths[e] == 128
                            and gvis[e][0] == kind
                            and gvis[e][1] == idx
                            and gvis[e][2] == gvis[e - 1][2] + 1
                            and offs[e] == offs[e - 1] + widths[e - 1]
                            and offs[e] + 128 <= (offs[s] // 512 + 1) * 512
                        ):
                            e += 1
                    d0 = offs[s]
                    d1 = offs[e - 1] + widths[e - 1]
                    lhsT = (
                        kt[:, idx * 128 : (idx + 1) * 128]
                        if kind == "local"
                        else ktv[:, idx * 128 : (idx + 1) * 128]
                    )
                    banks = range(d0 // 512, (d1 - 1) // 512 + 1)
                    nc.tensor.matmul(
                        stage[:, d0:d1],
                        lhsT,
                        qt[:, i0 * 128 : i0 * 128 + (d1 - d0)],
                        start=any(b not in seen_banks for b in banks),
                        stop=True,
                        skip_group_check=True,
                    )
                    seen_banks.update(banks)
                    s = e

                if len(pending_pv) >= pv_delay:
                    pending_pv.pop(0)()

                # --- exp for the group
                if "exp" not in ablate:
                    nc.scalar.activation(
                        out=ptt[:, 0:gw],
                        in_=stage[:, 0:gw],
                        func=mybir.ActivationFunctionType.Exp,
                    )

                # --- masks
                for s, (kind, idx, i) in enumerate(gvis):
                    if "masks" in ablate:
                        if kind == "vert":
                            v_idx += 1
                        continue
                    off, w = offs[s], widths[s]
                    sl = slice(off, off + w)
                    if kind == "local" and idx == i:
                        nc.vector.tensor_mul(ptt[:, sl], ptt[:, sl], tri)
                    elif kind == "local" and idx == i - 8:
                        nc.vector.tensor_mul(
                            ptt[:, sl], ptt[:, sl],
                            mstart[:, 0:64] if w == 64 else mstart,
                        )
                    elif kind == "vert":
                        for hh in range(2):
                            # skip halves whose validity scalar is 1.0 on
                            # every core (multiply-by-one is a no-op)
                            if vert_half_all_valid(idx, i, hh):
                                continue
                            hsl = slice(off + hh * 64, off + (hh + 1) * 64)
                            nc.vector.tensor_scalar_mul(
                                ptt[:, hsl], ptt[:, hsl], vs_ap(v_idx, hh)
                            )
                        v_idx += 1

                # --- PV matmuls + epilogue (deferred one group)
                def make_pv(gi, gvis, ptt, offs, widths):
                    def emit_pv():
                        if "pv" in ablate:
                            return
                        for s, (kind, idx, i) in enumerate(gvis):
                            g = gi * GROUP + s
                            pg = i // 3
                            if pg not in oacc_tiles:
                                oacc_tiles[pg] = oaccp.tile(
                                    [128, 3, D + 1], fp32, tag="oacc", name=f"oacc{pg}"
                                )
                            oacc = oacc_tiles[pg]
                            rhs = vaug[:, idx] if kind == "local" else vvaug[:, idx]
                            off, w = offs[s], widths[s]
                            nc.tensor.matmul(
                                oacc[:, i % 3] if w == 128
                                else oacc[0:64, i % 3],
                                ptt[:, off : off + w],
                                rhs,
                                start=(g == tile_first[i // 3]),
                                stop=(g == last[i]),
                                skip_group_check=True,
                            )
                            # epilogue once per oacc tile (after its last
                            # pair closes): a single DVE read of the PSUM
                            # bank, so PE's later PV writes to that bank are
                            # never serialized against mid-tile DVE reads.
                            # The final pair-group instead closes per pair so
                            # the very last store leaves as early as possible.
                            pg_pairs = [p for p in (3 * pg, 3 * pg + 1, 3 * pg + 2)
                                        if p < NPAIR]
                            if "epi" in ablate:
                                continue
                            if pg == last_pg and g == last[i]:
                                jj = i % 3
                                osb = obp.tile([128, D + 1], fp32, tag="osb1")
                                nc.vector.tensor_copy(osb[:], oacc[:, jj])
                                rd = rdp.tile([128, 1], fp32, tag="rd")
                                nc.vector.reciprocal(rd[:], osb[:, D : D + 1])
                                ob1 = obp.tile([128, 1, D], bf16, tag="ob1")
                                nc.vector.tensor_scalar_mul(
                                    ob1[:, 0], osb[:, 0:D], rd[:]
                                )
                                st_eng = nc.sync if i % 2 == 0 else nc.scalar
                                st_eng.dma_start(
                                    out=o_d[:, i : i + 1, :], in_=ob1[:]
                                )
                            elif (
                                pg != last_pg
                                and i == pg_pairs[-1]
                                and g == last[i]
                            ):
                                # single PSUM read frees the oacc bank fast
                                # (the next pg's PV start=True waits on it)
                                osb = obp.tile([128, 3, D + 1], fp32, tag="osb")
                                nc.vector.tensor_copy(osb[:], oacc[:])
                                ob3 = obp.tile([128, 3, D], bf16, tag="ob3")
                                for jj, pp in enumerate(pg_pairs):
                                    rd = rdp.tile([128, 1], fp32, tag="rd")
                                    nc.vector.reciprocal(
                                        rd[:], osb[:, jj, D : D + 1]
                                    )
                                    nc.vector.tensor_scalar_mul(
                                        ob3[:, jj], osb[:, jj, 0:D], rd[:]
                                    )
                                # one batched store per pair-group (bf16,
                                # (partition, pair, d) DRAM layout) on the
                                # two HWDGE queues, which are idle after the
                                # input loads and drain faster than SWDGE
                                p0 = pg_pairs[0]
                                st_eng = nc.sync if pg % 2 == 0 else nc.scalar
                                st_eng.dma_start(
                                    out=o_d[:, p0 : p0 + len(pg_pairs), :],
                                    in_=ob3[:, 0 : len(pg_pairs)],
                                )
                    return emit_pv

                pending_pv.append(make_pv(gi, gvis, ptt, offs, widths))
            for f in pending_pv:
                f()


def _get_program():
    global _PROGRAM
    if _PROGRAM is None:
        _PROGRAM = _build_program()
    return _PROGRAM


def _host_inputs(q, k, v, sm_scale):
    """Per-core input dicts (host-side shard + layout)."""
    q = np.asarray(q, dtype=np.float32)
    k = np.asarray(k, dtype=np.float32)
    v = np.asarray(v, dtype=np.float32)
    smv = float(np.asarray(sm_scale, dtype=np.float32))

    tri = np.zeros((128, 128), dtype=BF16)
    p = np.arange(128)
    tri[p[:, None] <= p[None, :]] = BF16(1.0)
    mstart = np.zeros((128, 128), dtype=BF16)
    mstart[64:, :64] = BF16(1.0)
    smsc = np.full((128, 1), smv, dtype=np.float32)

    vorder = vert_visit_order()
    ins = []
    for h in range(H):
        r = 7 - h
        qh, kh, vh = q[0, h], k[0, h], v[0, h]
        # fold sm_scale into q so the device exp needs no scale operand
        qt = np.ascontiguousarray(qh.T * smv).astype(BF16)
        kt = np.ascontiguousarray(kh.T).astype(BF16)
        vblocks = [8 * j + r for j in range(NVSLOT)]
        kv = np.concatenate([kh[b * BLK : (b + 1) * BLK] for b in vblocks], axis=0)
        ktv = np.ascontiguousarray(kv.T).astype(BF16)
        vaug = np.concatenate(
            [vh, np.ones((S, 1), np.float32)], axis=1
        ).astype(BF16)  # [4096, 129]
        vaug = np.ascontiguousarray(
            vaug.reshape(NPAIR, 128, D + 1).transpose(1, 0, 2)
        )  # [128, 32, 129]
        vv = np.concatenate([vh[b * BLK : (b + 1) * BLK] for b in vblocks], axis=0)
        vvaug = np.concatenate([vv, np.ones((NVSLOT * BLK, 1), np.float32)], axis=1)
        vvaug = np.ascontiguousarray(
            vvaug.astype(BF16).reshape(NVC, 128, D + 1).transpose(1, 0, 2)
        )  # [128, 3, 129]

        vsc = np.zeros((128, 48, 2), dtype=np.float32)
        for vi, (vc, i) in enumerate(vorder):
            for hh in range(2):
                qb = 2 * i + hh
                slot = 2 * vc + (p >= 64).astype(np.int64)  # per-partition slot
                kb = 8 * slot + r
                vsc[:, vi, hh] = (kb <= qb - 16).astype(np.float32)
        pkf = np.concatenate([smsc, vsc.reshape(128, 96)], axis=1)
        pkb = np.concatenate([tri, mstart], axis=1)
        ins.append(
            dict(
                qt=qt, kt=kt, ktv=ktv, vaug=vaug, vvaug=vvaug,
                pkf=pkf, pkb=pkb,
            )
        )
    return ins


def kernel(q, k, v, sm_scale):
    from concourse.bass_utils import run_bass_kernel_spmd

    nc = _get_program()
    ins = _host_inputs(q, k, v, sm_scale)
    res = run_bass_kernel_spmd(nc, ins, core_ids=list(range(H)))
    # o is bf16 [128, NPAIR, D] with (partition, pair, d) layout; unpermute
    out = np.stack(
        [
            np.ascontiguousarray(
                res.results[h]["o"].transpose(1, 0, 2)
            ).reshape(S, D)
            for h in range(H)
        ],
        axis=0,
    )[None]
    return out.astype(np.float32)



# revision 54
# speedup vs baseline: 1.0508x; 1.0229x over previous
"""Block-sparse attention (local + vertical-strided causal mask) on 8 TRN2 cores.

Sharding: one head per NeuronCore (H=8, n_cores=8).

Per-core device algorithm (head h, residue r = 7-h):
  The 4096x4096 score matrix is processed at 128x128 granularity:
  "pair" i = q block-rows (2i, 2i+1) (128 q tokens), "chunk" = 128 k tokens
  (2 mask blocks of 64). Local window -> chunks c in [i-8, i] of K itself;
  vertical-strided blocks -> host-gathered K_vert (6 blocks of 64, kb = 8j+r),
  processed as 3 chunks shared by all cores, with per-core validity applied
  as multiplicative 0/1 per-partition scalars (all-ones halves are skipped
  at build time). Vert visits are spread through the schedule (pair i's
  verts right before its closing chunk) to keep the DVE mask load even.

  S^T orientation: S^T[k,q] = kT_chunk.T @ qT_pair  (PE, bf16; sm_scale is
    folded into qT on the host). Window-start visits (c == i-8) only carry
    their valid first 64 q-cols when not tile-first.
  P^T = exp(S^T)                                     (ACT, one call per group)
  masks (triangle / window-start / vert validity)    (DVE)
  out[q,0:128] += P^T_chunk.T @ [V | 1]_chunk        (PE, PSUM-accumulated)
  col 128 of out = softmax denominator; normalize with per-partition
  reciprocal + tensor_scalar multiply into bf16, batched stores per
  pair-group ((partition, pair, d) DRAM layout; host unpermutes + casts).

Perf notes (why the structure looks like this):
  - Each dma_start costs ~650ns of issue time on its engine and ~2.5-4us of
    serialized per-instruction latency on its queue (~90 GB/s each), so
    inputs go out as a few large, need-ordered transfers balanced across
    the three DMA-capable queues (sync/scalar HWDGE + gpsimd SWDGE).
  - ~3.5us of throwaway matmuls on a memset tile warm the PE HAM clock gate
    (cold PE runs at 1.2 GHz instead of 2.4).
  - The steady-state window is Activation-bound (exp is 1 elem/cycle/lane
    at 1.2 GHz); PE and DVE run just below it.
"""

import numpy as np
import ml_dtypes

BF16 = ml_dtypes.bfloat16

H = 8
S = 4096
D = 128
BLK = 64
NB = S // BLK        # 64 block rows
NPAIR = NB // 2      # 32 row pairs
NVSLOT = 6           # usable vertical slots (kb = 8j + r <= 47)
NVC = NVSLOT // 2    # 3 vertical chunks
GROUP = 8            # PSUM staging slots per exp group (8 * 128 f32 = 2 banks)

NEG = -30000.0


def make_schedule():
    """Global ordered visit list. visit = (kind, idx, pair)
    kind "local": idx = chunk c (k blocks 2c, 2c+1), pairs i in [c, c+8]
    kind "vert":  idx = vc (K_vert slots 2vc, 2vc+1)
    Pair i's vert visits (all vc with 8*vc+8 <= i; chunks beyond that have no
    causally-valid content) are spread out: emitted right before pair i's
    closing local chunk c == i. This keeps the per-group DVE mask load even
    (clustered vert bursts back up the Vector engine, which stalls exp via
    ptt-buffer reuse)."""
    visits = []
    for c in range(NPAIR):
        for vc in range(NVC):
            if 8 * vc + 8 <= c:
                visits.append(("vert", vc, c))
        for i in range(c, min(c + 8, NPAIR - 1) + 1):
            visits.append(("local", c, i))
    return visits


def vert_visit_order():
    return [(vc_, i_) for (kind, vc_, i_) in make_schedule() if kind == "vert"]


def vert_half_all_valid(vc, i, hh):
    """True iff the validity scalar for this vert-visit half is 1.0 for every
    partition on every core (kb = 8*(2*vc + p//64) + r <= qb - 16 with
    qb = 2*i + hh, worst case r=7, slot 2*vc+1)."""
    return 8 * (2 * vc + 1) + 7 <= 2 * i + hh - 16


_PROGRAM = None


def _build_program(loop_n=None, ablate=(), pv_delay=3, group=GROUP, stage_bufs=2,
                   pt_bufs=None, exp_split=1, dma_split=16, qt_gpsimd=True,
                   ob_bufs=3, rd_bufs=4, vaug_gpsimd=False, store_sync=False):
    if pt_bufs is None:
        pt_bufs = pv_delay + 4
    """Build the SPMD program. loop_n: wrap the whole body (incl. input DMA)
    in an in-NEFF For loop with that trip count — used only for timing.
    ablate: subset of {"masks","pv","epi","exp"} — drop stages (timing only).
    pv_delay: groups of software-pipeline delay between S^T and PV.
    exp_split: number of ACT calls per group."""
    import contextlib
    import concourse.bass as bass
    import concourse.mybir as mybir
    import concourse.tile as tile
    from concourse import bacc

    fp32 = mybir.dt.float32
    bf16 = mybir.dt.bfloat16

    nc = bacc.Bacc("TRN2", target_bir_lowering=False, debug=False, num_devices=H)

    qt_d = nc.dram_tensor("qt", [D, S], bf16, kind="ExternalInput").ap()
    kt_d = nc.dram_tensor("kt", [D, S], bf16, kind="ExternalInput").ap()
    ktv_d = nc.dram_tensor("ktv", [D, NVSLOT * BLK], bf16, kind="ExternalInput").ap()
    vaug_d = nc.dram_tensor("vaug", [128, NPAIR, D + 1], bf16, kind="ExternalInput").ap()
    vvaug_d = nc.dram_tensor("vvaug", [128, NVC, D + 1], bf16, kind="ExternalInput").ap()
    # packed small tensors: one DMA each instead of four
    # pkf[:, 0] = unused, pkf[:, 1:97] = vs.reshape(128, 96)
    # (sm_scale is folded into qt on the host, so exp uses scale=1.0 imm)
    pkf_d = nc.dram_tensor("pkf", [128, 97], fp32, kind="ExternalInput").ap()
    # pkb[:, 0:128] = tri, pkb[:, 128:256] = mstart
    pkb_d = nc.dram_tensor("pkb", [128, 256], bf16, kind="ExternalInput").ap()
    # output in (partition, pair, d) layout, bf16; host unpermutes + casts
    o_d = nc.dram_tensor("o", [128, NPAIR, D], bf16, kind="ExternalOutput").ap()

    visits = make_schedule()
    # first/last visit index per pair
    first = {}
    last = {}
    for g, (kind, idx, i) in enumerate(visits):
        first.setdefault(i, g)
        last[i] = g
    # PSUM start_tensor_calc zeroes the full 2KB bank (zero-region), so only
    # the first matmul touching an oacc tile may carry start=True.
    tile_first = {}
    for g, (kind, idx, i) in enumerate(visits):
        tile_first.setdefault(i // 3, g)
    with tile.TileContext(nc) as tc:
        with (
            tc.tile_pool(name="big", bufs=1) as big,
            tc.tile_pool(name="stage", bufs=stage_bufs, space="PSUM") as stagep,
            tc.tile_pool(name="oacc", bufs=4, space="PSUM") as oaccp,
            tc.tile_pool(name="pt", bufs=pt_bufs) as ptp,
            tc.tile_pool(name="ob", bufs=ob_bufs) as obp,
            tc.tile_pool(name="rd", bufs=rd_bufs) as rdp,
        ):
            if loop_n is not None:
                loop_cm = tc.For_i(
                    0,
                    loop_n,
                    hint_engines=(
                        mybir.EngineType.PE,
                        mybir.EngineType.DVE,
                        mybir.EngineType.Activation,
                        mybir.EngineType.Pool,
                        mybir.EngineType.SP,
                    ),
                )
            else:
                loop_cm = contextlib.nullcontext()
            with loop_cm:
                _emit_body(nc, tc, locals(), frozenset(ablate),
                           pv_delay=pv_delay, group=group, exp_split=exp_split,
                           dma_split=dma_split, qt_gpsimd=qt_gpsimd,
                           vaug_gpsimd=vaug_gpsimd, store_sync=store_sync)
    nc.compile()
    return nc


def _emit_body(nc, tc, env, ablate=frozenset(), pv_delay=1, group=GROUP,
               exp_split=1, dma_split=16, qt_gpsimd=False, vaug_gpsimd=False,
               store_sync=False):
    GROUP = group
    import concourse.mybir as mybir

    fp32 = mybir.dt.float32
    bf16 = mybir.dt.bfloat16
    big, stagep, oaccp, ptp, obp, rdp = (
        env["big"], env["stagep"], env["oaccp"], env["ptp"], env["obp"], env["rdp"]
    )
    qt_d, kt_d, ktv_d, vaug_d, vvaug_d, pkf_d, pkb_d, o_d = (
        env["qt_d"], env["kt_d"], env["ktv_d"], env["vaug_d"], env["vvaug_d"],
        env["pkf_d"], env["pkb_d"], env["o_d"],
    )
    visits, first, last, tile_first = (
        env["visits"], env["first"], env["last"], env["tile_first"]
    )
    n_groups = (len(visits) + GROUP - 1) // GROUP
    if True:
        if True:
            qt = big.tile([D, S], bf16)
            kt = big.tile([D, S], bf16)
            ktv = big.tile([D, NVSLOT * BLK], bf16)
            vaug = big.tile([128, NPAIR, D + 1], bf16)
            vvaug = big.tile([128, NVC, D + 1], bf16)
            pkf = big.tile([128, 97], fp32)
            pkb = big.tile([128, 256], bf16)
            tri = pkb[:, 0:128]
            mstart = pkb[:, 128:256]

            def vs_ap(vi, hh):  # pkf col 1 + 2*vi + hh  (was vs[:, vi, hh])
                c0 = 1 + 2 * vi + hh
                return pkf[:, c0 : c0 + 1]

            # Input DMA: few big transfers, spread over the three DMA-capable
            # engines (sync/scalar HWDGE, gpsimd SWDGE), in first-need order.
            # Each dma_start costs ~650ns of engine issue time and each queue
            # sustains only ~90-150 GB/s, so balance bytes across all three
            # queues and order by first use.
            # Completion is serialized per queue at ~2.5-4us per DMA
            # *instruction* (fixed HWDGE/SWDGE overhead dominates for <1MB),
            # so the chain ORDER on each queue sets availability times.
            nc.sync.dma_start(out=kt[:, 0:512], in_=kt_d[:, 0:512])
            nc.sync.dma_start(out=pkb[:], in_=pkb_d[:])
            nc.sync.dma_start(out=kt[:, 512:1536], in_=kt_d[:, 512:1536])
            nc.sync.dma_start(out=kt[:, 1536:2560], in_=kt_d[:, 1536:2560])
            nc.sync.dma_start(out=kt[:, 2560:S], in_=kt_d[:, 2560:S])
            nc.scalar.dma_start(out=qt[:, 0:512], in_=qt_d[:, 0:512])
            nc.scalar.dma_start(out=qt[:, 1280:1536], in_=qt_d[:, 1280:1536])
            nc.scalar.dma_start(out=qt[:, 1536:2560], in_=qt_d[:, 1536:2560])
            nc.scalar.dma_start(out=ktv[:], in_=ktv_d[:])
            nc.scalar.dma_start(out=vvaug[:], in_=vvaug_d[:])
            nc.scalar.dma_start(out=qt[:, 2560:S], in_=qt_d[:, 2560:S])
            nc.gpsimd.dma_start(out=qt[:, 512:1280], in_=qt_d[:, 512:1280])
            nc.gpsimd.dma_start(out=vaug[:, 0:8], in_=vaug_d[:, 0:8])
            nc.gpsimd.dma_start(out=vaug[:, 8:20], in_=vaug_d[:, 8:20])
            nc.gpsimd.dma_start(out=vaug[:, 20:NPAIR], in_=vaug_d[:, 20:NPAIR])
            nc.gpsimd.dma_start(out=pkf[:], in_=pkf_d[:])

            # PE warm-up: ~3.5us of throwaway matmuls on a memset tile (no
            # DMA dependency) so the HAM clock gate opens (K=8/8) before the
            # real S^T stream begins; without this the first ~10 groups run
            # at 1.2 GHz.
            wsrc = big.tile([128, 256], bf16)
            nc.vector.memset(wsrc[:], 1.0)
            wtile = stagep.tile([128, GROUP * 128], fp32, tag="stage")
            for _w in range(24):
                nc.tensor.matmul(
                    wtile[:, 0:256], wsrc[:, 0:128], wsrc[:],
                    start=True, stop=True, skip_group_check=True,
                )

            oacc_tiles = {}  # pair-group (i//3) -> psum tile [128, 3, 129]
            v_idx = 0  # running vertical-visit index (matches host vs layout)
            pending_pv = []  # software pipeline: PV of group gi-d emitted
            # after S^T of group gi so PE streams while ACT/DVE process gi-d

            last_pg = (NPAIR - 1) // 3
            for gi in range(n_groups):
                gvis = visits[gi * GROUP : (gi + 1) * GROUP]
                n = len(gvis)
                stage = stagep.tile([128, GROUP * 128], fp32, tag="stage")
                ptt = ptp.tile([128, GROUP * 128], bf16, tag="pt")

                # Per-visit stage widths: window-start visits (idx == i-8)
                # only have valid content in their first 64 q-cols (qblock
                # 2i); use a 64-wide slot unless the visit is tile-first (a
                # 64-partition PV start=True might not zero the whole bank).
                widths = []
                for s, (kind, idx, i) in enumerate(gvis):
                    g = gi * GROUP + s
                    narrow = (
                        kind == "local" and idx == i - 8
                        and tile_first[i // 3] != g
                    )
                    widths.append(64 if narrow else 128)
                offs = []
                off = 0
                for w in widths:
                    if w == 128 and off // 512 != (off + 127) // 512:
                        off = (off // 512 + 1) * 512  # don't straddle a bank
                    offs.append(off)
                    off += w
                gw = off

                # --- S^T matmuls, batched over runs of consecutive pairs
                # sharing one k-chunk, split at PSUM bank (512 f32) bounds.
                # start=True only on the first run per bank (bank zero-region).
                s = 0
                seen_banks = set()
                while s < n:
                    kind, idx, i0 = gvis[s]
                    e = s + 1
                    if widths[s] == 128:
                        while (
                            e < n
                            and widths[e] == 128
                            and gvis[e][0] == kind
                            and gvis[e][1] == idx
                            and gvis[e][2] == gvis[e - 1][2] + 1
                            and offs[e] == offs[e - 1] + widths[e - 1]
                            and offs[e] + 128 <= (offs[s] // 512 + 1) * 512
                        ):
                            e += 1
                    d0 = offs[s]
                    d1 = offs[e - 1] + widths[e - 1]
                    lhsT = (
                        kt[:, idx * 128 : (idx + 1) * 128]
                        if kind == "local"
                        else ktv[:, idx * 128 : (idx + 1) * 128]
                    )
                    banks = range(d0 // 512, (d1 - 1) // 512 + 1)
                    nc.tensor.matmul(
                        stage[:, d0:d1],
                        lhsT,
                        qt[:, i0 * 128 : i0 * 128 + (d1 - d0)],
                        start=any(b not in seen_banks for b in banks),
                        stop=True,
                        skip_group_check=True,
                    )
                    seen_banks.update(banks)
                    s = e

                if len(pending_pv) >= pv_delay:
                    pending_pv.pop(0)()

                # --- exp for the group
                if "exp" not in ablate:
                    nc.scalar.activation(
                        out=ptt[:, 0:gw],
                        in_=stage[:, 0:gw],
                        func=mybir.ActivationFunctionType.Exp,
                    )

                # --- masks
                for s, (kind, idx, i) in enumerate(gvis):
                    if "masks" in ablate:
                        if kind == "vert":
                            v_idx += 1
                        continue
                    off, w = offs[s], widths[s]
                    sl = slice(off, off + w)
                    if kind == "local" and idx == i:
                        nc.vector.tensor_mul(ptt[:, sl], ptt[:, sl], tri)
                    elif kind == "local" and idx == i - 8:
                        nc.vector.tensor_mul(
                            ptt[:, sl], ptt[:, sl],
                            mstart[:, 0:64] if w == 64 else mstart,
                        )
                    elif kind == "vert":
                        for hh in range(2):
                            # skip halves whose validity scalar is 1.0 on
                            # every core (multiply-by-one is a no-op)
                            if vert_half_all_valid(idx, i, hh):
                                continue
                            hsl = slice(off + hh * 64, off + (hh + 1) * 64)
                            nc.vector.tensor_scalar_mul(
                                ptt[:, hsl], ptt[:, hsl], vs_ap(v_idx, hh)
                            )
                        v_idx += 1

                # --- PV matmuls + epilogue (deferred one group)
                def make_pv(gi, gvis, ptt, offs, widths):
                    def emit_pv():
                        if "pv" in ablate:
                            return
                        for s, (kind, idx, i) in enumerate(gvis):
                            g = gi * GROUP + s
                            pg = i // 3
                            if pg not in oacc_tiles:
                                oacc_tiles[pg] = oaccp.tile(
                                    [128, 3, D + 1], fp32, tag="oacc", name=f"oacc{pg}"
                                )
                            oacc = oacc_tiles[pg]
                            rhs = vaug[:, idx] if kind == "local" else vvaug[:, idx]
                            off, w = offs[s], widths[s]
                            nc.tensor.matmul(
                                oacc[:, i % 3] if w == 128
                                else oacc[0:64, i % 3],
                                ptt[:, off : off + w],
                                rhs,
                                start=(g == tile_first[i // 3]),
                                stop=(g == last[i]),
                                skip_group_check=True,
                            )
                            # epilogue once per oacc tile (after its last
                            # pair closes): a single DVE read of the PSUM
                            # bank, so PE's later PV writes to that bank are
                            # never serialized against mid-tile DVE reads.
                            # The final pair-group instead closes per pair so
                            # the very last store leaves as early as possible.
                            pg_pairs = [p for p in (3 * pg, 3 * pg + 1, 3 * pg + 2)
                                        if p < NPAIR]
                            if "epi" in ablate:
                                continue
                            if pg == last_pg and g == last[i]:
                                jj = i % 3
                                osb = obp.tile([128, D + 1], fp32, tag="osb1")
                                nc.vector.tensor_copy(osb[:], oacc[:, jj])
                                rd = rdp.tile([128, 1], fp32, tag="rd")
                                nc.vector.reciprocal(rd[:], osb[:, D : D + 1])
                                ob1 = obp.tile([128, 1, D], bf16, tag="ob1")
                                nc.vector.tensor_scalar_mul(
                                    ob1[:, 0], osb[:, 0:D], rd[:]
                                )
                                st_eng = nc.sync if i % 2 == 0 else nc.scalar
                                st_eng.dma_start(
                                    out=o_d[:, i : i + 1, :], in_=ob1[:]
                                )
                            elif (
                                pg != last_pg
                                and i == pg_pairs[-1]
                                and g == last[i]
                            ):
                                # single PSUM read frees the oacc bank fast
                                # (the next pg's PV start=True waits on it)
                                osb = obp.tile([128, 3, D + 1], fp32, tag="osb")
                                nc.vector.tensor_copy(osb[:], oacc[:])
                                ob3 = obp.tile([128, 3, D], bf16, tag="ob3")
                                for jj, pp in enumerate(pg_pairs):
                                    rd = rdp.tile([128, 1], fp32, tag="rd")
                                    nc.vector.reciprocal(
                                        rd[:], osb[:, jj, D : D + 1]
                                    )
                                    nc.vector.tensor_scalar_mul(
                                        ob3[:, jj], osb[:, jj, 0:D], rd[:]
                                    )
                                # one batched store per pair-group (bf16,
                                # (partition, pair, d) DRAM layout) on the
                                # two HWDGE queues, which are idle after the
                                # input loads and drain faster than SWDGE
                                p0 = pg_pairs[0]
                                st_eng = nc.sync if pg % 2 == 0 else nc.scalar
                                st_eng.dma_start(
                                    out=o_d[:, p0 : p0 + len(pg_pairs), :],
                                    in_=ob3[:, 0 : len(pg_pairs)],
                                )
                    return emit_pv

                pending_pv.append(make_pv(gi, gvis, ptt, offs, widths))
            for f in pending_pv:
                f()


def _get_program():
    global _PROGRAM
    if _PROGRAM is None:
        _PROGRAM = _build_program()
    return _PROGRAM


def _host_inputs(q, k, v, sm_scale):
    """Per-core input dicts (host-side shard + layout)."""
    q = np.asarray(q, dtype=np.float32)
    k = np.asarray(k, dtype=np.float32)
    v = np.asarray(v, dtype=np.float32)
    smv = float(np.asarray(sm_scale, dtype=np.float32))

    tri = np.zeros((128, 128), dtype=BF16)
    p = np.arange(128)
    tri[p[:, None] <= p[None, :]] = BF16(1.0)
    mstart = np.zeros((128, 128), dtype=BF16)
    mstart[64:, :64] = BF16(1.0)
    smsc = np.full((128, 1), smv, dtype=np.float32)

    vorder = vert_visit_order()
    ins = []
    for h in range(H):
        r = 7 - h
        qh, kh, vh = q[0, h], k[0, h], v[0, h]
        # fold sm_scale into q so the device exp needs no scale operand
        qt = np.ascontiguousarray(qh.T * smv).astype(BF16)
        kt = np.ascontiguousarray(kh.T).astype(BF16)
        vblocks = [8 * j + r for j in range(NVSLOT)]
        kv = np.concatenate([kh[b * BLK : (b + 1) * BLK] for b in vblocks], axis=0)
        ktv = np.ascontiguousarray(kv.T).astype(BF16)
        vaug = np.concatenate(
            [vh, np.ones((S, 1), np.float32)], axis=1
        ).astype(BF16)  # [4096, 129]
        vaug = np.ascontiguousarray(
            vaug.reshape(NPAIR, 128, D + 1).transpose(1, 0, 2)
        )  # [128, 32, 129]
        vv = np.concatenate([vh[b * BLK : (b + 1) * BLK] for b in vblocks], axis=0)
        vvaug = np.concatenate([vv, np.ones((NVSLOT * BLK, 1), np.float32)], axis=1)
        vvaug = np.ascontiguousarray(
            vvaug.astype(BF16).reshape(NVC, 128, D + 1).transpose(1, 0, 2)
        )  # [128, 3, 129]

        vsc = np.zeros((128, 48, 2), dtype=np.float32)
        for vi, (vc, i) in enumerate(vorder):
            for hh in range(2):
                qb = 2 * i + hh
                slot = 2 * vc + (p >= 64).astype(np.int64)  # per-partition slot
                kb = 8 * slot + r
                vsc[:, vi, hh] = (kb <= qb - 16).astype(np.float32)
        pkf = np.concatenate([smsc, vsc.reshape(128, 96)], axis=1)
        pkb = np.concatenate([tri, mstart], axis=1)
        ins.append(
            dict(
                qt=qt, kt=kt, ktv=ktv, vaug=vaug, vvaug=vvaug,
                pkf=pkf, pkb=pkb,
            )
        )
    return ins


def kernel(q, k, v, sm_scale):
    from concourse.bass_utils import run_bass_kernel_spmd

    nc = _get_program()
    ins = _host_inputs(q, k, v, sm_scale)
    res = run_bass_kernel_spmd(nc, ins, core_ids=list(range(H)))
    # o is bf16 [128, NPAIR, D] with (partition, pair, d) layout; unpermute
    out = np.stack(
        [
            np.ascontiguousarray(
                res.results[h]["o"].transpose(1, 0, 2)
            ).reshape(S, D)
            for h in range(H)
        ],
        axis=0,
    )[None]
    return out.astype(np.float32)



# revision 55
# speedup vs baseline: 1.0757x; 1.0237x over previous
"""Block-sparse attention (local + vertical-strided causal mask) on 8 TRN2 cores.

Sharding: one head per NeuronCore (H=8, n_cores=8).

Per-core device algorithm (head h, residue r = 7-h):
  The 4096x4096 score matrix is processed at 128x128 granularity:
  "pair" i = q block-rows (2i, 2i+1) (128 q tokens), "chunk" = 128 k tokens
  (2 mask blocks of 64). Local window -> chunks c in [i-8, i] of K itself;
  vertical-strided blocks -> host-gathered K_vert (6 blocks of 64, kb = 8j+r),
  processed as 3 chunks shared by all cores, with per-core validity applied
  as multiplicative 0/1 per-partition scalars (all-ones halves are skipped
  at build time). Vert visits are spread through the schedule (pair i's
  verts right before its closing chunk) to keep the DVE mask load even.

  S^T orientation: S^T[k,q] = kT_chunk.T @ qT_pair  (PE, bf16; sm_scale is
    folded into qT on the host). Window-start visits (c == i-8) only carry
    their valid first 64 q-cols when not tile-first.
  P^T = exp(S^T)                                     (ACT, one call per group)
  masks (triangle / window-start / vert validity)    (DVE)
  out[q,0:128] += P^T_chunk.T @ [V | 1]_chunk        (PE, PSUM-accumulated)
  col 128 of out = softmax denominator; normalize with per-partition
  reciprocal + tensor_scalar multiply into bf16, batched stores per
  pair-group ((partition, pair, d) DRAM layout; host unpermutes + casts).

Perf notes (why the structure looks like this):
  - Each dma_start costs ~650ns of issue time on its engine and ~2.5-4us of
    serialized per-instruction latency on its queue (~90 GB/s each), so
    inputs go out as a few large, need-ordered transfers balanced across
    the three DMA-capable queues (sync/scalar HWDGE + gpsimd SWDGE).
  - ~3.5us of throwaway matmuls on a memset tile warm the PE HAM clock gate
    (cold PE runs at 1.2 GHz instead of 2.4).
  - The steady-state window is Activation-bound (exp is 1 elem/cycle/lane
    at 1.2 GHz); PE and DVE run just below it.
"""

import numpy as np
import ml_dtypes

BF16 = ml_dtypes.bfloat16

H = 8
S = 4096
D = 128
BLK = 64
NB = S // BLK        # 64 block rows
NPAIR = NB // 2      # 32 row pairs
NVSLOT = 6           # usable vertical slots (kb = 8j + r <= 47)
NVC = NVSLOT // 2    # 3 vertical chunks
GROUP = 8            # PSUM staging slots per exp group (8 * 128 f32 = 2 banks)

NEG = -30000.0


def make_schedule():
    """Global ordered visit list. visit = (kind, idx, pair)
    kind "local": idx = chunk c (k blocks 2c, 2c+1), pairs i in [c, c+8]
    kind "vert":  idx = vc (K_vert slots 2vc, 2vc+1)
    Pair i's vert visits (all vc with 8*vc+8 <= i; chunks beyond that have no
    causally-valid content) are spread out: emitted right before pair i's
    closing local chunk c == i. This keeps the per-group DVE mask load even
    (clustered vert bursts back up the Vector engine, which stalls exp via
    ptt-buffer reuse)."""
    visits = []
    for c in range(NPAIR):
        for vc in range(NVC):
            if 8 * vc + 8 <= c:
                visits.append(("vert", vc, c))
        for i in range(c, min(c + 8, NPAIR - 1) + 1):
            visits.append(("local", c, i))
    return visits


def vert_visit_order():
    return [(vc_, i_) for (kind, vc_, i_) in make_schedule() if kind == "vert"]


def vert_half_all_valid(vc, i, hh):
    """True iff the validity scalar for this vert-visit half is 1.0 for every
    partition on every core (kb = 8*(2*vc + p//64) + r <= qb - 16 with
    qb = 2*i + hh, worst case r=7, slot 2*vc+1)."""
    return 8 * (2 * vc + 1) + 7 <= 2 * i + hh - 16


_PROGRAM = None


def _build_program(loop_n=None, ablate=(), pv_delay=3, group=GROUP, stage_bufs=2,
                   pt_bufs=None, exp_split=1, dma_split=16, qt_gpsimd=True,
                   ob_bufs=3, rd_bufs=4, vaug_gpsimd=False, store_sync=False):
    if pt_bufs is None:
        pt_bufs = pv_delay + 4
    """Build the SPMD program. loop_n: wrap the whole body (incl. input DMA)
    in an in-NEFF For loop with that trip count — used only for timing.
    ablate: subset of {"masks","pv","epi","exp"} — drop stages (timing only).
    pv_delay: groups of software-pipeline delay between S^T and PV.
    exp_split: number of ACT calls per group."""
    import contextlib
    import concourse.bass as bass
    import concourse.mybir as mybir
    import concourse.tile as tile
    from concourse import bacc

    fp32 = mybir.dt.float32
    bf16 = mybir.dt.bfloat16

    nc = bacc.Bacc("TRN2", target_bir_lowering=False, debug=False, num_devices=H)

    qt_d = nc.dram_tensor("qt", [D, S], bf16, kind="ExternalInput").ap()
    kt_d = nc.dram_tensor("kt", [D, S], bf16, kind="ExternalInput").ap()
    ktv_d = nc.dram_tensor("ktv", [D, NVSLOT * BLK], bf16, kind="ExternalInput").ap()
    vaug_d = nc.dram_tensor("vaug", [128, NPAIR, D + 1], bf16, kind="ExternalInput").ap()
    vvaug_d = nc.dram_tensor("vvaug", [128, NVC, D + 1], bf16, kind="ExternalInput").ap()
    # packed small tensors: one DMA each instead of four
    # pkf[:, 0] = unused, pkf[:, 1:97] = vs.reshape(128, 96)
    # (sm_scale is folded into qt on the host, so exp uses scale=1.0 imm)
    pkf_d = nc.dram_tensor("pkf", [128, 97], fp32, kind="ExternalInput").ap()
    # pkb[:, 0:128] = tri, pkb[:, 128:256] = mstart
    pkb_d = nc.dram_tensor("pkb", [128, 256], bf16, kind="ExternalInput").ap()
    # output in (partition, pair, d) layout, bf16; host unpermutes + casts
    o_d = nc.dram_tensor("o", [128, NPAIR, D], bf16, kind="ExternalOutput").ap()

    visits = make_schedule()
    # first/last visit index per pair
    first = {}
    last = {}
    for g, (kind, idx, i) in enumerate(visits):
        first.setdefault(i, g)
        last[i] = g
    # PSUM start_tensor_calc zeroes the full 2KB bank (zero-region), so only
    # the first matmul touching an oacc tile may carry start=True.
    tile_first = {}
    for g, (kind, idx, i) in enumerate(visits):
        tile_first.setdefault(i // 3, g)
    with tile.TileContext(nc) as tc:
        with (
            tc.tile_pool(name="big", bufs=1) as big,
            tc.tile_pool(name="stage", bufs=stage_bufs, space="PSUM") as stagep,
            tc.tile_pool(name="oacc", bufs=4, space="PSUM") as oaccp,
            tc.tile_pool(name="pt", bufs=pt_bufs) as ptp,
            tc.tile_pool(name="ob", bufs=ob_bufs) as obp,
            tc.tile_pool(name="rd", bufs=rd_bufs) as rdp,
        ):
            if loop_n is not None:
                loop_cm = tc.For_i(
                    0,
                    loop_n,
                    hint_engines=(
                        mybir.EngineType.PE,
                        mybir.EngineType.DVE,
                        mybir.EngineType.Activation,
                        mybir.EngineType.Pool,
                        mybir.EngineType.SP,
                    ),
                )
            else:
                loop_cm = contextlib.nullcontext()
            with loop_cm:
                _emit_body(nc, tc, locals(), frozenset(ablate),
                           pv_delay=pv_delay, group=group, exp_split=exp_split,
                           dma_split=dma_split, qt_gpsimd=qt_gpsimd,
                           vaug_gpsimd=vaug_gpsimd, store_sync=store_sync)
    nc.compile()
    return nc


def _emit_body(nc, tc, env, ablate=frozenset(), pv_delay=1, group=GROUP,
               exp_split=1, dma_split=16, qt_gpsimd=False, vaug_gpsimd=False,
               store_sync=False):
    GROUP = group
    import concourse.mybir as mybir

    fp32 = mybir.dt.float32
    bf16 = mybir.dt.bfloat16
    big, stagep, oaccp, ptp, obp, rdp = (
        env["big"], env["stagep"], env["oaccp"], env["ptp"], env["obp"], env["rdp"]
    )
    qt_d, kt_d, ktv_d, vaug_d, vvaug_d, pkf_d, pkb_d, o_d = (
        env["qt_d"], env["kt_d"], env["ktv_d"], env["vaug_d"], env["vvaug_d"],
        env["pkf_d"], env["pkb_d"], env["o_d"],
    )
    visits, first, last, tile_first = (
        env["visits"], env["first"], env["last"], env["tile_first"]
    )
    n_groups = (len(visits) + GROUP - 1) // GROUP
    if True:
        if True:
            qt = big.tile([D, S], bf16)
            kt = big.tile([D, S], bf16)
            ktv = big.tile([D, NVSLOT * BLK], bf16)
            vaug = big.tile([128, NPAIR, D + 1], bf16)
            vvaug = big.tile([128, NVC, D + 1], bf16)
            pkf = big.tile([128, 97], fp32)
            pkb = big.tile([128, 256], bf16)
            tri = pkb[:, 0:128]
            mstart = pkb[:, 128:256]

            def vs_ap(vi, hh):  # pkf col 1 + 2*vi + hh  (was vs[:, vi, hh])
                c0 = 1 + 2 * vi + hh
                return pkf[:, c0 : c0 + 1]

            # Input DMA: few big transfers, spread over the three DMA-capable
            # engines (sync/scalar HWDGE, gpsimd SWDGE), in first-need order.
            # Each dma_start costs ~650ns of engine issue time and each queue
            # sustains only ~90-150 GB/s, so balance bytes across all three
            # queues and order by first use.
            # Completion is serialized per queue at ~2.5-4us per DMA
            # *instruction* (fixed HWDGE/SWDGE overhead dominates for <1MB),
            # so the chain ORDER on each queue sets availability times.
            nc.sync.dma_start(out=kt[:, 0:512], in_=kt_d[:, 0:512])
            nc.sync.dma_start(out=pkb[:], in_=pkb_d[:])
            nc.sync.dma_start(out=kt[:, 512:1536], in_=kt_d[:, 512:1536])
            nc.sync.dma_start(out=kt[:, 1536:2560], in_=kt_d[:, 1536:2560])
            nc.sync.dma_start(out=kt[:, 2560:S], in_=kt_d[:, 2560:S])
            nc.scalar.dma_start(out=qt[:, 0:512], in_=qt_d[:, 0:512])
            nc.scalar.dma_start(out=qt[:, 1280:1536], in_=qt_d[:, 1280:1536])
            nc.scalar.dma_start(out=qt[:, 1536:2560], in_=qt_d[:, 1536:2560])
            nc.scalar.dma_start(out=ktv[:], in_=ktv_d[:])
            nc.scalar.dma_start(out=vvaug[:], in_=vvaug_d[:])
            nc.scalar.dma_start(out=qt[:, 2560:S], in_=qt_d[:, 2560:S])
            nc.gpsimd.dma_start(out=qt[:, 512:1280], in_=qt_d[:, 512:1280])
            nc.gpsimd.dma_start(out=vaug[:, 0:8], in_=vaug_d[:, 0:8])
            nc.gpsimd.dma_start(out=vaug[:, 8:20], in_=vaug_d[:, 8:20])
            nc.gpsimd.dma_start(out=vaug[:, 20:NPAIR], in_=vaug_d[:, 20:NPAIR])
            nc.gpsimd.dma_start(out=pkf[:], in_=pkf_d[:])

            # PE warm-up: ~3.5us of throwaway matmuls on a memset tile (no
            # DMA dependency) so the HAM clock gate opens (K=8/8) before the
            # real S^T stream begins; without this the first ~10 groups run
            # at 1.2 GHz.
            wsrc = big.tile([128, 256], bf16)
            nc.vector.memset(wsrc[:], 1.0)
            wtile = stagep.tile([128, GROUP * 128], fp32, tag="stage")
            for _w in range(24):
                nc.tensor.matmul(
                    wtile[:, 0:256], wsrc[:, 0:128], wsrc[:],
                    start=True, stop=True, skip_group_check=True,
                )

            oacc_tiles = {}  # pair-group (i//3) -> psum tile [128, 3, 129]
            v_idx = 0  # running vertical-visit index (matches host vs layout)
            pending_pv = []  # software pipeline: PV of group gi-d emitted
            # after S^T of group gi so PE streams while ACT/DVE process gi-d

            last_pg = (NPAIR - 1) // 3
            for gi in range(n_groups):
                gvis = visits[gi * GROUP : (gi + 1) * GROUP]
                n = len(gvis)
                stage = stagep.tile([128, GROUP * 128], fp32, tag="stage")
                ptt = ptp.tile([128, GROUP * 128], bf16, tag="pt")

                # Per-visit stage widths: window-start visits (idx == i-8)
                # only have valid content in their first 64 q-cols (qblock
                # 2i); use a 64-wide slot unless the visit is tile-first (a
                # 64-partition PV start=True might not zero the whole bank).
                widths = []
                for s, (kind, idx, i) in enumerate(gvis):
                    g = gi * GROUP + s
                    narrow = (
                        kind == "local" and idx == i - 8
                        and tile_first[i // 3] != g
                    )
                    widths.append(64 if narrow else 128)
                offs = []
                off = 0
                for w in widths:
                    if w == 128 and off // 512 != (off + 127) // 512:
                        off = (off // 512 + 1) * 512  # don't straddle a bank
                    offs.append(off)
                    off += w
                gw = off

                # --- S^T matmuls, batched over runs of consecutive pairs
                # sharing one k-chunk, split at PSUM bank (512 f32) bounds.
                # start=True only on the first run per bank (bank zero-region).
                s = 0
                seen_banks = set()
                while s < n:
                    kind, idx, i0 = gvis[s]
                    e = s + 1
                    if widths[s] == 128:
                        while (
                            e < n
                            and widths[e] == 128
                            and gvis[e][0] == kind
                            and gvis[e][1] == idx
                            and gvis[e][2] == gvis[e - 1][2] + 1
                            and offs[e] == offs[e - 1] + widths[e - 1]
                            and offs[e] + 128 <= (offs[s] // 512 + 1) * 512
                        ):
                            e += 1
                    d0 = offs[s]
                    d1 = offs[e - 1] + widths[e - 1]
                    lhsT = (
                        kt[:, idx * 128 : (idx + 1) * 128]
                        if kind == "local"
                        else ktv[:, idx * 128 : (idx + 1) * 128]
                    )
                    banks = range(d0 // 512, (d1 - 1) // 512 + 1)
                    nc.tensor.matmul(
                        stage[:, d0:d1],
                        lhsT,
                        qt[:, i0 * 128 : i0 * 128 + (d1 - d0)],
                        start=any(b not in seen_banks for b in banks),
                        stop=True,
                        skip_group_check=True,
                    )
                    seen_banks.update(banks)
                    s = e

                if len(pending_pv) >= pv_delay:
                    pending_pv.pop(0)()

                # --- exp for the group
                if "exp" not in ablate:
                    nc.scalar.activation(
                        out=ptt[:, 0:gw],
                        in_=stage[:, 0:gw],
                        func=mybir.ActivationFunctionType.Exp,
                    )

                # --- masks
                for s, (kind, idx, i) in enumerate(gvis):
                    if "masks" in ablate:
                        if kind == "vert":
                            v_idx += 1
                        continue
                    off, w = offs[s], widths[s]
                    sl = slice(off, off + w)
                    if kind == "local" and idx == i:
                        nc.vector.tensor_mul(ptt[:, sl], ptt[:, sl], tri)
                    elif kind == "local" and idx == i - 8:
                        nc.vector.tensor_mul(
                            ptt[:, sl], ptt[:, sl],
                            mstart[:, 0:64] if w == 64 else mstart,
                        )
                    elif kind == "vert":
                        for hh in range(2):
                            # skip halves whose validity scalar is 1.0 on
                            # every core (multiply-by-one is a no-op)
                            if vert_half_all_valid(idx, i, hh):
                                continue
                            hsl = slice(off + hh * 64, off + (hh + 1) * 64)
                            nc.vector.tensor_scalar_mul(
                                ptt[:, hsl], ptt[:, hsl], vs_ap(v_idx, hh)
                            )
                        v_idx += 1

                # --- PV matmuls + epilogue (deferred one group)
                def make_pv(gi, gvis, ptt, offs, widths):
                    def emit_pv():
                        if "pv" in ablate:
                            return
                        for s, (kind, idx, i) in enumerate(gvis):
                            g = gi * GROUP + s
                            pg = i // 3
                            if pg not in oacc_tiles:
                                oacc_tiles[pg] = oaccp.tile(
                                    [128, 3, D + 1], fp32, tag="oacc", name=f"oacc{pg}"
                                )
                            oacc = oacc_tiles[pg]
                            rhs = vaug[:, idx] if kind == "local" else vvaug[:, idx]
                            off, w = offs[s], widths[s]
                            nc.tensor.matmul(
                                oacc[:, i % 3] if w == 128
                                else oacc[0:64, i % 3],
                                ptt[:, off : off + w],
                                rhs,
                                start=(g == tile_first[i // 3]),
                                stop=(g == last[i]),
                                skip_group_check=True,
                            )
                            # epilogue once per oacc tile (after its last
                            # pair closes): a single DVE read of the PSUM
                            # bank, so PE's later PV writes to that bank are
                            # never serialized against mid-tile DVE reads.
                            # The final pair-group instead closes per pair so
                            # the very last store leaves as early as possible.
                            pg_pairs = [p for p in (3 * pg, 3 * pg + 1, 3 * pg + 2)
                                        if p < NPAIR]
                            if "epi" in ablate:
                                continue
                            if pg == last_pg and g == last[i]:
                                # read PSUM directly: these banks are never
                                # recycled, so skipping the staging copy just
                                # shortens the serial tail chain
                                jj = i % 3
                                rd = rdp.tile([128, 1], fp32, tag="rd")
                                nc.vector.reciprocal(
                                    rd[:], oacc[:, jj, D : D + 1]
                                )
                                ob1 = obp.tile([128, 1, D], bf16, tag="ob1")
                                nc.vector.tensor_scalar_mul(
                                    ob1[:, 0], oacc[:, jj, 0:D], rd[:]
                                )
                                st_eng = nc.sync if i % 2 == 0 else nc.scalar
                                st_eng.dma_start(
                                    out=o_d[:, i : i + 1, :], in_=ob1[:]
                                )
                            elif (
                                pg != last_pg
                                and i == pg_pairs[-1]
                                and g == last[i]
                            ):
                                # single PSUM read frees the oacc bank fast
                                # (the next pg's PV start=True waits on it)
                                osb = obp.tile([128, 3, D + 1], fp32, tag="osb")
                                nc.vector.tensor_copy(osb[:], oacc[:])
                                ob3 = obp.tile([128, 3, D], bf16, tag="ob3")
                                for jj, pp in enumerate(pg_pairs):
                                    rd = rdp.tile([128, 1], fp32, tag="rd")
                                    nc.vector.reciprocal(
                                        rd[:], osb[:, jj, D : D + 1]
                                    )
                                    nc.vector.tensor_scalar_mul(
                                        ob3[:, jj], osb[:, jj, 0:D], rd[:]
                                    )
                                # one batched store per pair-group (bf16,
                                # (partition, pair, d) DRAM layout) on the
                                # two HWDGE queues, which are idle after the
                                # input loads and drain faster than SWDGE
                                p0 = pg_pairs[0]
                                st_eng = nc.sync if pg % 2 == 0 else nc.scalar
                                st_eng.dma_start(
                                    out=o_d[:, p0 : p0 + len(pg_pairs), :],
                                    in_=ob3[:, 0 : len(pg_pairs)],
                                )
                    return emit_pv

                pending_pv.append(make_pv(gi, gvis, ptt, offs, widths))
            for f in pending_pv:
                f()


def _get_program():
    global _PROGRAM
    if _PROGRAM is None:
        _PROGRAM = _build_program()
    return _PROGRAM


def _host_inputs(q, k, v, sm_scale):
    """Per-core input dicts (host-side shard + layout)."""
    q = np.asarray(q, dtype=np.float32)
    k = np.asarray(k, dtype=np.float32)
    v = np.asarray(v, dtype=np.float32)
    smv = float(np.asarray(sm_scale, dtype=np.float32))

    tri = np.zeros((128, 128), dtype=BF16)
    p = np.arange(128)
    tri[p[:, None] <= p[None, :]] = BF16(1.0)
    mstart = np.zeros((128, 128), dtype=BF16)
    mstart[64:, :64] = BF16(1.0)
    smsc = np.full((128, 1), smv, dtype=np.float32)

    vorder = vert_visit_order()
    ins = []
    for h in range(H):
        r = 7 - h
        qh, kh, vh = q[0, h], k[0, h], v[0, h]
        # fold sm_scale into q so the device exp needs no scale operand
        qt = np.ascontiguousarray(qh.T * smv).astype(BF16)
        kt = np.ascontiguousarray(kh.T).astype(BF16)
        vblocks = [8 * j + r for j in range(NVSLOT)]
        kv = np.concatenate([kh[b * BLK : (b + 1) * BLK] for b in vblocks], axis=0)
        ktv = np.ascontiguousarray(kv.T).astype(BF16)
        vaug = np.concatenate(
            [vh, np.ones((S, 1), np.float32)], axis=1
        ).astype(BF16)  # [4096, 129]
        vaug = np.ascontiguousarray(
            vaug.reshape(NPAIR, 128, D + 1).transpose(1, 0, 2)
        )  # [128, 32, 129]
        vv = np.concatenate([vh[b * BLK : (b + 1) * BLK] for b in vblocks], axis=0)
        vvaug = np.concatenate([vv, np.ones((NVSLOT * BLK, 1), np.float32)], axis=1)
        vvaug = np.ascontiguousarray(
            vvaug.astype(BF16).reshape(NVC, 128, D + 1).transpose(1, 0, 2)
        )  # [128, 3, 129]

        vsc = np.zeros((128, 48, 2), dtype=np.float32)
        for vi, (vc, i) in enumerate(vorder):
            for hh in range(2):
                qb = 2 * i + hh
                slot = 2 * vc + (p >= 64).astype(np.int64)  # per-partition slot
                kb = 8 * slot + r
                vsc[:, vi, hh] = (kb <= qb - 16).astype(np.float32)
        pkf = np.concatenate([smsc, vsc.reshape(128, 96)], axis=1)
        pkb = np.concatenate([tri, mstart], axis=1)
        ins.append(
            dict(
                qt=qt, kt=kt, ktv=ktv, vaug=vaug, vvaug=vvaug,
                pkf=pkf, pkb=pkb,
            )
        )
    return ins


def kernel(q, k, v, sm_scale):
    from concourse.bass_utils import run_bass_kernel_spmd

    nc = _get_program()
    ins = _host_inputs(q, k, v, sm_scale)
    res = run_bass_kernel_spmd(nc, ins, core_ids=list(range(H)))
    # o is bf16 [128, NPAIR, D] with (partition, pair, d) layout; unpermute
    out = np.stack(
        [
            np.ascontiguousarray(
                res.results[h]["o"].transpose(1, 0, 2)
            ).reshape(S, D)
            for h in range(H)
        ],
        axis=0,
    )[None]
    return out.astype(np.float32)



# revision 57
# speedup vs baseline: 1.0945x; 1.0175x over previous
"""Block-sparse attention (local + vertical-strided causal mask) on 8 TRN2 cores.

Sharding: one head per NeuronCore (H=8, n_cores=8).

Per-core device algorithm (head h, residue r = 7-h):
  The 4096x4096 score matrix is processed at 128x128 granularity:
  "pair" i = q block-rows (2i, 2i+1) (128 q tokens), "chunk" = 128 k tokens
  (2 mask blocks of 64). Local window -> chunks c in [i-8, i] of K itself;
  vertical-strided blocks -> host-gathered K_vert (6 blocks of 64, kb = 8j+r),
  processed as 3 chunks shared by all cores, with per-core validity applied
  as multiplicative 0/1 per-partition scalars (all-ones halves are skipped
  at build time). Vert visits are spread through the schedule (pair i's
  verts right before its closing chunk) to keep the DVE mask load even.

  S^T orientation: S^T[k,q] = kT_chunk.T @ qT_pair  (PE, bf16; sm_scale is
    folded into qT on the host). Window-start visits (c == i-8) only carry
    their valid first 64 q-cols when not tile-first.
  P^T = exp(S^T)                                     (ACT, one call per group)
  masks (triangle / window-start / vert validity)    (DVE)
  out[q,0:128] += P^T_chunk.T @ [V | 1]_chunk        (PE, PSUM-accumulated)
  col 128 of out = softmax denominator; normalize with per-partition
  reciprocal + tensor_scalar multiply into bf16, batched stores per
  pair-group ((partition, pair, d) DRAM layout; host unpermutes + casts).

Perf notes (why the structure looks like this):
  - Each dma_start costs ~650ns of issue time on its engine and ~2.5-4us of
    serialized per-instruction latency on its queue (~90 GB/s each), so
    inputs go out as a few large, need-ordered transfers balanced across
    the three DMA-capable queues (sync/scalar HWDGE + gpsimd SWDGE).
  - ~3.5us of throwaway matmuls on a memset tile warm the PE HAM clock gate
    (cold PE runs at 1.2 GHz instead of 2.4).
  - The steady-state window is Activation-bound (exp is 1 elem/cycle/lane
    at 1.2 GHz); PE and DVE run just below it.
"""

import numpy as np
import ml_dtypes

BF16 = ml_dtypes.bfloat16

H = 8
S = 4096
D = 128
BLK = 64
NB = S // BLK        # 64 block rows
NPAIR = NB // 2      # 32 row pairs
NVSLOT = 6           # usable vertical slots (kb = 8j + r <= 47)
NVC = NVSLOT // 2    # 3 vertical chunks
GROUP = 8            # PSUM staging slots per exp group (8 * 128 f32 = 2 banks)

NEG = -30000.0


def make_schedule():
    """Global ordered visit list. visit = (kind, idx, pair)
    kind "local": idx = chunk c (k blocks 2c, 2c+1), pairs i in [c, c+8]
    kind "vert":  idx = vc (K_vert slots 2vc, 2vc+1)
    Pair i's vert visits (all vc with 8*vc+8 <= i; chunks beyond that have no
    causally-valid content) are spread out: emitted right before pair i's
    closing local chunk c == i. This keeps the per-group DVE mask load even
    (clustered vert bursts back up the Vector engine, which stalls exp via
    ptt-buffer reuse)."""
    visits = []
    for c in range(NPAIR):
        for vc in range(NVC):
            if 8 * vc + 8 <= c:
                visits.append(("vert", vc, c))
        for i in range(c, min(c + 8, NPAIR - 1) + 1):
            visits.append(("local", c, i))
    return visits


def vert_visit_order():
    return [(vc_, i_) for (kind, vc_, i_) in make_schedule() if kind == "vert"]


def vert_half_all_valid(vc, i, hh):
    """True iff the validity scalar for this vert-visit half is 1.0 for every
    partition on every core (kb = 8*(2*vc + p//64) + r <= qb - 16 with
    qb = 2*i + hh, worst case r=7, slot 2*vc+1)."""
    return 8 * (2 * vc + 1) + 7 <= 2 * i + hh - 16


_PROGRAM = None


def _build_program(loop_n=None, ablate=(), pv_delay=3, group=GROUP, stage_bufs=2,
                   pt_bufs=None, exp_split=1, dma_split=16, qt_gpsimd=True,
                   ob_bufs=3, rd_bufs=4, vaug_gpsimd=False, store_sync=False):
    if pt_bufs is None:
        pt_bufs = pv_delay + 4
    """Build the SPMD program. loop_n: wrap the whole body (incl. input DMA)
    in an in-NEFF For loop with that trip count — used only for timing.
    ablate: subset of {"masks","pv","epi","exp"} — drop stages (timing only).
    pv_delay: groups of software-pipeline delay between S^T and PV.
    exp_split: number of ACT calls per group."""
    import contextlib
    import concourse.bass as bass
    import concourse.mybir as mybir
    import concourse.tile as tile
    from concourse import bacc

    fp32 = mybir.dt.float32
    bf16 = mybir.dt.bfloat16

    nc = bacc.Bacc("TRN2", target_bir_lowering=False, debug=False, num_devices=H)

    qt_d = nc.dram_tensor("qt", [D, S], bf16, kind="ExternalInput").ap()
    kt_d = nc.dram_tensor("kt", [D, S], bf16, kind="ExternalInput").ap()
    ktv_d = nc.dram_tensor("ktv", [D, NVSLOT * BLK], bf16, kind="ExternalInput").ap()
    vaug_d = nc.dram_tensor("vaug", [128, NPAIR, D + 1], bf16, kind="ExternalInput").ap()
    vvaug_d = nc.dram_tensor("vvaug", [128, NVC, D + 1], bf16, kind="ExternalInput").ap()
    # packed small tensors: one DMA each instead of four
    # pkf[:, 0] = unused, pkf[:, 1:97] = vs.reshape(128, 96)
    # (sm_scale is folded into qt on the host, so exp uses scale=1.0 imm)
    pkf_d = nc.dram_tensor("pkf", [128, 97], fp32, kind="ExternalInput").ap()
    # pkb[:, 0:128] = tri, pkb[:, 128:256] = mstart
    pkb_d = nc.dram_tensor("pkb", [128, 256], bf16, kind="ExternalInput").ap()
    # output in (partition, pair, d) layout, bf16; host unpermutes + casts
    o_d = nc.dram_tensor("o", [128, NPAIR, D], bf16, kind="ExternalOutput").ap()

    visits = make_schedule()
    # first/last visit index per pair
    first = {}
    last = {}
    for g, (kind, idx, i) in enumerate(visits):
        first.setdefault(i, g)
        last[i] = g
    # PSUM start_tensor_calc zeroes the full 2KB bank (zero-region), so only
    # the first matmul touching an oacc tile may carry start=True.
    tile_first = {}
    for g, (kind, idx, i) in enumerate(visits):
        tile_first.setdefault(i // 3, g)
    with tile.TileContext(nc) as tc:
        with (
            tc.tile_pool(name="big", bufs=1) as big,
            tc.tile_pool(name="stage", bufs=stage_bufs, space="PSUM") as stagep,
            tc.tile_pool(name="oacc", bufs=4, space="PSUM") as oaccp,
            tc.tile_pool(name="pt", bufs=pt_bufs) as ptp,
            tc.tile_pool(name="ob", bufs=ob_bufs) as obp,
            tc.tile_pool(name="rd", bufs=rd_bufs) as rdp,
        ):
            if loop_n is not None:
                loop_cm = tc.For_i(
                    0,
                    loop_n,
                    hint_engines=(
                        mybir.EngineType.PE,
                        mybir.EngineType.DVE,
                        mybir.EngineType.Activation,
                        mybir.EngineType.Pool,
                        mybir.EngineType.SP,
                    ),
                )
            else:
                loop_cm = contextlib.nullcontext()
            with loop_cm:
                _emit_body(nc, tc, locals(), frozenset(ablate),
                           pv_delay=pv_delay, group=group, exp_split=exp_split,
                           dma_split=dma_split, qt_gpsimd=qt_gpsimd,
                           vaug_gpsimd=vaug_gpsimd, store_sync=store_sync)
    nc.compile()
    return nc


def _emit_body(nc, tc, env, ablate=frozenset(), pv_delay=1, group=GROUP,
               exp_split=1, dma_split=16, qt_gpsimd=False, vaug_gpsimd=False,
               store_sync=False):
    GROUP = group
    import concourse.mybir as mybir

    fp32 = mybir.dt.float32
    bf16 = mybir.dt.bfloat16
    big, stagep, oaccp, ptp, obp, rdp = (
        env["big"], env["stagep"], env["oaccp"], env["ptp"], env["obp"], env["rdp"]
    )
    qt_d, kt_d, ktv_d, vaug_d, vvaug_d, pkf_d, pkb_d, o_d = (
        env["qt_d"], env["kt_d"], env["ktv_d"], env["vaug_d"], env["vvaug_d"],
        env["pkf_d"], env["pkb_d"], env["o_d"],
    )
    visits, first, last, tile_first = (
        env["visits"], env["first"], env["last"], env["tile_first"]
    )
    n_groups = (len(visits) + GROUP - 1) // GROUP
    if True:
        if True:
            qt = big.tile([D, S], bf16)
            kt = big.tile([D, S], bf16)
            ktv = big.tile([D, NVSLOT * BLK], bf16)
            vaug = big.tile([128, NPAIR, D + 1], bf16)
            vvaug = big.tile([128, NVC, D + 1], bf16)
            pkf = big.tile([128, 97], fp32)
            pkb = big.tile([128, 256], bf16)
            tri = pkb[:, 0:128]
            mstart = pkb[:, 128:256]

            def vs_ap(vi, hh):  # pkf col 1 + 2*vi + hh  (was vs[:, vi, hh])
                c0 = 1 + 2 * vi + hh
                return pkf[:, c0 : c0 + 1]

            # Input DMA: few big transfers, spread over the three DMA-capable
            # engines (sync/scalar HWDGE, gpsimd SWDGE), in first-need order.
            # Each dma_start costs ~650ns of engine issue time and each queue
            # sustains only ~90-150 GB/s, so balance bytes across all three
            # queues and order by first use.
            # Completion is serialized per queue at ~2.5-4us per DMA
            # *instruction* (fixed HWDGE/SWDGE overhead dominates for <1MB),
            # so the chain ORDER on each queue sets availability times.
            nc.sync.dma_start(out=kt[:, 0:512], in_=kt_d[:, 0:512])
            nc.sync.dma_start(out=pkb[:], in_=pkb_d[:])
            nc.sync.dma_start(out=kt[:, 512:1536], in_=kt_d[:, 512:1536])
            nc.sync.dma_start(out=kt[:, 1536:2560], in_=kt_d[:, 1536:2560])
            nc.sync.dma_start(out=kt[:, 2560:S], in_=kt_d[:, 2560:S])
            nc.scalar.dma_start(out=qt[:, 0:512], in_=qt_d[:, 0:512])
            nc.scalar.dma_start(out=qt[:, 1280:1536], in_=qt_d[:, 1280:1536])
            nc.scalar.dma_start(out=qt[:, 1536:2560], in_=qt_d[:, 1536:2560])
            nc.scalar.dma_start(out=ktv[:], in_=ktv_d[:])
            nc.scalar.dma_start(out=vvaug[:], in_=vvaug_d[:])
            nc.scalar.dma_start(out=qt[:, 2560:S], in_=qt_d[:, 2560:S])
            nc.gpsimd.dma_start(out=qt[:, 512:1280], in_=qt_d[:, 512:1280])
            nc.gpsimd.dma_start(out=vaug[:, 0:8], in_=vaug_d[:, 0:8])
            nc.gpsimd.dma_start(out=vaug[:, 8:20], in_=vaug_d[:, 8:20])
            nc.gpsimd.dma_start(out=vaug[:, 20:NPAIR], in_=vaug_d[:, 20:NPAIR])
            nc.gpsimd.dma_start(out=pkf[:], in_=pkf_d[:])

            # PE warm-up: ~3.5us of throwaway matmuls on a memset tile (no
            # DMA dependency) so the HAM clock gate opens (K=8/8) before the
            # real S^T stream begins; without this the first ~10 groups run
            # at 1.2 GHz.
            wsrc = big.tile([128, 256], bf16)
            nc.vector.memset(wsrc[:], 1.0)
            wtile = stagep.tile([128, GROUP * 128], fp32, tag="stage")
            for _w in range(24):
                nc.tensor.matmul(
                    wtile[:, 0:256], wsrc[:, 0:128], wsrc[:],
                    start=True, stop=True, skip_group_check=True,
                )

            oacc_tiles = {}  # pair-group (i//3) -> psum tile [128, 3, 129]
            v_idx = 0  # running vertical-visit index (matches host vs layout)
            pending_pv = []  # software pipeline: PV of group gi-d emitted
            # after S^T of group gi so PE streams while ACT/DVE process gi-d

            last_pg = (NPAIR - 1) // 3
            for gi in range(n_groups):
                gvis = visits[gi * GROUP : (gi + 1) * GROUP]
                n = len(gvis)
                stage = stagep.tile([128, GROUP * 128], fp32, tag="stage")
                ptt = ptp.tile([128, GROUP * 128], bf16, tag="pt")

                # Per-visit stage widths: window-start visits (idx == i-8)
                # only have valid content in their first 64 q-cols (qblock
                # 2i); use a 64-wide slot unless the visit is tile-first (a
                # 64-partition PV start=True might not zero the whole bank).
                widths = []
                for s, (kind, idx, i) in enumerate(gvis):
                    g = gi * GROUP + s
                    narrow = (
                        kind == "local" and idx == i - 8
                        and tile_first[i // 3] != g
                    )
                    widths.append(64 if narrow else 128)
                offs = []
                off = 0
                for w in widths:
                    if w == 128 and off // 512 != (off + 127) // 512:
                        off = (off // 512 + 1) * 512  # don't straddle a bank
                    offs.append(off)
                    off += w
                gw = off

                # --- S^T matmuls, batched over runs of consecutive pairs
                # sharing one k-chunk, split at PSUM bank (512 f32) bounds.
                # start=True only on the first run per bank (bank zero-region).
                s = 0
                seen_banks = set()
                while s < n:
                    kind, idx, i0 = gvis[s]
                    e = s + 1
                    if widths[s] == 128:
                        while (
                            e < n
                            and widths[e] == 128
                            and gvis[e][0] == kind
                            and gvis[e][1] == idx
                            and gvis[e][2] == gvis[e - 1][2] + 1
                            and offs[e] == offs[e - 1] + widths[e - 1]
                            and offs[e] + 128 <= (offs[s] // 512 + 1) * 512
                        ):
                            e += 1
                    d0 = offs[s]
                    d1 = offs[e - 1] + widths[e - 1]
                    lhsT = (
                        kt[:, idx * 128 : (idx + 1) * 128]
                        if kind == "local"
                        else ktv[:, idx * 128 : (idx + 1) * 128]
                    )
                    banks = range(d0 // 512, (d1 - 1) // 512 + 1)
                    nc.tensor.matmul(
                        stage[:, d0:d1],
                        lhsT,
                        qt[:, i0 * 128 : i0 * 128 + (d1 - d0)],
                        start=any(b not in seen_banks for b in banks),
                        stop=True,
                        skip_group_check=True,
                    )
                    seen_banks.update(banks)
                    s = e

                if len(pending_pv) >= pv_delay:
                    pending_pv.pop(0)()

                # --- exp for the group
                if "exp" not in ablate:
                    nc.scalar.activation(
                        out=ptt[:, 0:gw],
                        in_=stage[:, 0:gw],
                        func=mybir.ActivationFunctionType.Exp,
                    )

                # --- masks
                for s, (kind, idx, i) in enumerate(gvis):
                    if "masks" in ablate:
                        if kind == "vert":
                            v_idx += 1
                        continue
                    off, w = offs[s], widths[s]
                    sl = slice(off, off + w)
                    if kind == "local" and idx == i:
                        nc.vector.tensor_mul(ptt[:, sl], ptt[:, sl], tri)
                    elif kind == "local" and idx == i - 8:
                        nc.vector.tensor_mul(
                            ptt[:, sl], ptt[:, sl],
                            mstart[:, 0:64] if w == 64 else mstart,
                        )
                    elif kind == "vert":
                        for hh in range(2):
                            # skip halves whose validity scalar is 1.0 on
                            # every core (multiply-by-one is a no-op)
                            if vert_half_all_valid(idx, i, hh):
                                continue
                            hsl = slice(off + hh * 64, off + (hh + 1) * 64)
                            nc.vector.tensor_scalar_mul(
                                ptt[:, hsl], ptt[:, hsl], vs_ap(v_idx, hh)
                            )
                        v_idx += 1

                # --- PV matmuls + epilogue (deferred one group)
                def make_pv(gi, gvis, ptt, offs, widths):
                    def emit_pv():
                        if "pv" in ablate:
                            return
                        for s, (kind, idx, i) in enumerate(gvis):
                            g = gi * GROUP + s
                            pg = i // 3
                            if pg not in oacc_tiles:
                                oacc_tiles[pg] = oaccp.tile(
                                    [128, 3, D + 1], fp32, tag="oacc", name=f"oacc{pg}"
                                )
                            oacc = oacc_tiles[pg]
                            rhs = vaug[:, idx] if kind == "local" else vvaug[:, idx]
                            off, w = offs[s], widths[s]
                            nc.tensor.matmul(
                                oacc[:, i % 3] if w == 128
                                else oacc[0:64, i % 3],
                                ptt[:, off : off + w],
                                rhs,
                                start=(g == tile_first[i // 3]),
                                stop=(g == last[i]),
                                skip_group_check=True,
                            )
                            # epilogue once per oacc tile (after its last
                            # pair closes): a single DVE read of the PSUM
                            # bank, so PE's later PV writes to that bank are
                            # never serialized against mid-tile DVE reads.
                            # The final pair-group instead closes per pair so
                            # the very last store leaves as early as possible.
                            pg_pairs = [p for p in (3 * pg, 3 * pg + 1, 3 * pg + 2)
                                        if p < NPAIR]
                            if "epi" in ablate:
                                continue
                            if pg == last_pg and g == last[i]:
                                # read PSUM directly: these banks are never
                                # recycled, so skipping the staging copy just
                                # shortens the serial tail chain
                                jj = i % 3
                                rd = rdp.tile([128, 1], fp32, tag="rd")
                                nc.vector.reciprocal(
                                    rd[:], oacc[:, jj, D : D + 1]
                                )
                                ob1 = obp.tile([128, 1, D], bf16, tag="ob1")
                                nc.vector.tensor_scalar_mul(
                                    ob1[:, 0], oacc[:, jj, 0:D], rd[:]
                                )
                                st_eng = nc.sync if i % 2 == 0 else nc.scalar
                                st_eng.dma_start(
                                    out=o_d[:, i : i + 1, :], in_=ob1[:]
                                )
                            elif (
                                pg != last_pg
                                and i == pg_pairs[-1]
                                and g == last[i]
                            ):
                                # single PSUM read frees the oacc bank fast
                                # (the next pg's PV start=True waits on it)
                                osb = obp.tile([128, 3, D + 1], fp32, tag="osb")
                                nc.vector.tensor_copy(osb[:], oacc[:])
                                ob3 = obp.tile([128, 3, D], bf16, tag="ob3")
                                for jj, pp in enumerate(pg_pairs):
                                    rd = rdp.tile([128, 1], fp32, tag="rd")
                                    nc.vector.reciprocal(
                                        rd[:], osb[:, jj, D : D + 1]
                                    )
                                    nc.vector.tensor_scalar_mul(
                                        ob3[:, jj], osb[:, jj, 0:D], rd[:]
                                    )
                                # one batched store per pair-group (bf16,
                                # (partition, pair, d) DRAM layout) on the
                                # two HWDGE queues, which are idle after the
                                # input loads and drain faster than SWDGE
                                p0 = pg_pairs[0]
                                st_eng = nc.sync if pg % 2 == 0 else nc.scalar
                                st_eng.dma_start(
                                    out=o_d[:, p0 : p0 + len(pg_pairs), :],
                                    in_=ob3[:, 0 : len(pg_pairs)],
                                )
                    return emit_pv

                pending_pv.append(make_pv(gi, gvis, ptt, offs, widths))
            for f in pending_pv:
                f()


def _get_program():
    global _PROGRAM
    if _PROGRAM is None:
        _PROGRAM = _build_program()
    return _PROGRAM


def _host_inputs(q, k, v, sm_scale):
    """Per-core input dicts (host-side shard + layout)."""
    q = np.asarray(q, dtype=np.float32)
    k = np.asarray(k, dtype=np.float32)
    v = np.asarray(v, dtype=np.float32)
    smv = float(np.asarray(sm_scale, dtype=np.float32))

    tri = np.zeros((128, 128), dtype=BF16)
    p = np.arange(128)
    tri[p[:, None] <= p[None, :]] = BF16(1.0)
    mstart = np.zeros((128, 128), dtype=BF16)
    mstart[64:, :64] = BF16(1.0)
    smsc = np.full((128, 1), smv, dtype=np.float32)

    vorder = vert_visit_order()
    ins = []
    for h in range(H):
        r = 7 - h
        qh, kh, vh = q[0, h], k[0, h], v[0, h]
        # fold sm_scale into q so the device exp needs no scale operand
        qt = np.ascontiguousarray(qh.T * smv).astype(BF16)
        kt = np.ascontiguousarray(kh.T).astype(BF16)
        vblocks = [8 * j + r for j in range(NVSLOT)]
        kv = np.concatenate([kh[b * BLK : (b + 1) * BLK] for b in vblocks], axis=0)
        ktv = np.ascontiguousarray(kv.T).astype(BF16)
        vaug = np.concatenate(
            [vh, np.ones((S, 1), np.float32)], axis=1
        ).astype(BF16)  # [4096, 129]
        vaug = np.ascontiguousarray(
            vaug.reshape(NPAIR, 128, D + 1).transpose(1, 0, 2)
        )  # [128, 32, 129]
        vv = np.concatenate([vh[b * BLK : (b + 1) * BLK] for b in vblocks], axis=0)
        vvaug = np.concatenate([vv, np.ones((NVSLOT * BLK, 1), np.float32)], axis=1)
        vvaug = np.ascontiguousarray(
            vvaug.astype(BF16).reshape(NVC, 128, D + 1).transpose(1, 0, 2)
        )  # [128, 3, 129]

        vsc = np.zeros((128, 48, 2), dtype=np.float32)
        for vi, (vc, i) in enumerate(vorder):
            for hh in range(2):
                qb = 2 * i + hh
                slot = 2 * vc + (p >= 64).astype(np.int64)  # per-partition slot
                kb = 8 * slot + r
                vsc[:, vi, hh] = (kb <= qb - 16).astype(np.float32)
        pkf = np.concatenate([smsc, vsc.reshape(128, 96)], axis=1)
        pkb = np.concatenate([tri, mstart], axis=1)
        ins.append(
            dict(
                qt=qt, kt=kt, ktv=ktv, vaug=vaug, vvaug=vvaug,
                pkf=pkf, pkb=pkb,
            )
        )
    return ins


def kernel(q, k, v, sm_scale):
    from concourse.bass_utils import run_bass_kernel_spmd

    nc = _get_program()
    ins = _host_inputs(q, k, v, sm_scale)
    res = run_bass_kernel_spmd(nc, ins, core_ids=list(range(H)))
    # o is bf16 [128, NPAIR, D] with (partition, pair, d) layout; unpermute
    out = np.stack(
        [
            np.ascontiguousarray(
                res.results[h]["o"].transpose(1, 0, 2)
            ).reshape(S, D)
            for h in range(H)
        ],
        axis=0,
    )[None]
    return out.astype(np.float32)

